# revision 25
# baseline (speedup 1.0000x reference)
"""Trainium2 Bass kernel v3 for nn_AdaptiveFourierTransformGateLayer.

Data-parallel over batch: 8 cores x 256 rows. Per core:

  Host prep: xw = x * fc_w (scale+layout only), reflection-fold over l:
    xe[b,c,l'] = xw[b,l',c] + xw[b,2048-l',c]   (l'=1..1023; l'=0 -> xw[b,0,c])
    xo[b,c,l'] = xw[b,l',c] - xw[b,2048-l',c]   (l'=0 -> 0)
    hm[b] = sum_c xw[b,1024,c]                  (midpoint row)
  Folding halves the DFT to 1024x1024 half-matrices (C even / S odd).
  fc_b is dropped: AC-bin column sums of the DFT are exactly zero.
  Everything streamed in fp16, laid out [l'-part, c, b] so the channel
  tree-reduce lands directly in DFT-ready [l', b] layout (no transposes).

  Device:
  A: c-tree reduction (DVE, fp16 2x mode) -> HeT/HoT [l'-part, b].
  B: DFT chase: per f-chunk PSUM bank holds xr | xi halves; fp16 matmuls
     Ch-chunk^T @ HeT / Sh-chunk^T @ HoT accumulate as l'-chunks arrive.
     Midpoint rank-1 term alt(f) x hm(b) closes xr. 7 banks chase, f-chunk
     7 runs as a second wave after bank 0 evacuates. Evac to fp16
     xr/xi/xs (xs = xr+xi for Karatsuba).
  C: layer 1 via 3-matmul Karatsuba complex product:
       m1 = (xr+xi)@W1r, m2 = xi@(W1r+W1i), m3 = xr@(W1i-W1r)
       o1r = relu(m1-m2+b1r), o1i = relu(m1+m3+b1i), o1s = o1r+o1i
     m1|m2 share a PSUM bank, m3 in a second bank. Transposed dataflow
     (stationary = weight chunk, moving = activations [128,256]).
  D: layer 2 same Karatsuba shape; amp = sqrt((m1-m2+b2r)^2+(m1+m3+b2i)^2)
     -> ampT f32r. Gate matmuls (ampT @ wgn, f32r) chased per f-chunk.
  E: noisy top-3 softmax -> gates (small DVE/Act chain only).
"""
import sys
import types
import contextlib
import ctypes

import numpy as np

if "/opt/trn_rl_repo" not in sys.path:
    sys.path.insert(0, "/opt/trn_rl_repo")

# ---------------------------------------------------------------------------
# NTFF trace hook shim (only used when trace=True; harmless otherwise)
# ---------------------------------------------------------------------------


def _install_trace_shim():
    if "antenv.axon_hooks" in sys.modules:
        return
    so_path = "/opt/axon/libaxon_pjrt.so"

    def _mk():
        try:
            lib = ctypes.CDLL(so_path)
        except OSError:
            return None
        if not hasattr(lib, "axon_start_nrt_profile"):
            return None
        lib.axon_start_nrt_profile.argtypes = [
            ctypes.POINTER(ctypes.c_int64),
            ctypes.c_size_t,
        ]
        lib.axon_start_nrt_profile.restype = ctypes.c_int64
        lib.axon_stop_nrt_profile.argtypes = [ctypes.c_char_p]
        lib.axon_stop_nrt_profile.restype = ctypes.c_int64

        @contextlib.contextmanager
        def _hook(output_dir, device_ids):
            import jax

            jax.devices()
            if device_ids:
                ids = (ctypes.c_int64 * len(device_ids))(*device_ids)
                rc = lib.axon_start_nrt_profile(ids, len(device_ids))
            else:
                rc = lib.axon_start_nrt_profile(None, 0)
            if rc != 0:
                raise RuntimeError(f"axon_start_nrt_profile rc={rc}")
            try:
                yield
            finally:
                n = lib.axon_stop_nrt_profile(str(output_dir).encode())
                print(f"profile: {n} file(s) written to {output_dir}", file=sys.stderr)

        return _hook

    mod = types.ModuleType("antenv.axon_hooks")
    mod._hook = _mk()
    mod.get_axon_ntff_profile_hook = lambda: mod._hook
    mod.set_axon_ntff_profile_hook = lambda h: setattr(mod, "_hook", h)
    sys.modules["antenv.axon_hooks"] = mod
    try:
        import antenv

        antenv.axon_hooks = mod
    except ImportError:
        pass


_install_trace_shim()

import concourse.tile as tile  # noqa: E402
from concourse import bacc, mybir  # noqa: E402
from concourse.bass_utils import run_bass_kernel_spmd  # noqa: E402

# ---------------------------------------------------------------------------
# Problem constants (hardcoded)
# ---------------------------------------------------------------------------
B = 2048
L = 2048
CH = 16
F = 1024  # num freqs (rfft bins 1..1024)
FH = 4096  # hidden
E = 88  # num experts
NOISE_EPS = 0.01
_DEBUG_DUMP = False
NCORES = 8
BL = B // NCORES  # 256 rows per core
F32R = mybir.dt.float32r
F32 = mybir.dt.float32
FP16 = mybir.dt.float16

KARA1 = False  # Karatsuba in layer 1 (off: fp16 weight combos + relu
               # boundary flips cost 3x accuracy; plain 4-matmul instead)
KARA2 = True  # ... in layer 2

ADD = mybir.AluOpType.add
MULT = mybir.AluOpType.mult
AF = mybir.ActivationFunctionType


def rnd11(x):
    """Round-to-nearest keeping 11 mantissa bits (hardware f32r rounding)."""
    a = np.ascontiguousarray(x, np.float32)
    ai = a.view(np.uint32)
    return ((ai + np.uint32(1 << 11)) & np.uint32(0xFFFFF000)).view(np.float32)


def _build_program(training: bool):
    nc = bacc.Bacc("TRN2", target_bir_lowering=False, debug=False, num_devices=NCORES)

    # [eo, lc, p(l'), c, b] - host pre-transposed so tree-reduce -> [l', b]
    xeo_d = nc.dram_tensor("xeo", [2, 8, 128, CH, 256], FP16,
                           kind="ExternalInput").ap()
    # [p(l'), kc, fc, 128 f-cols]
    chs_d = nc.dram_tensor("chs", [128, 8, 8, 128], FP16, kind="ExternalInput").ap()
    shs_d = nc.dram_tensor("shs", [128, 8, 8, 128], FP16, kind="ExternalInput").ap()
    # [hg, p(f), fc, h-cols 512]
    k1_d = nc.dram_tensor("k1", [8, 128, 8, 512], FP16, kind="ExternalInput").ap()
    k2_d = nc.dram_tensor("k2", [8, 128, 8, 512], FP16, kind="ExternalInput").ap()
    k3_d = nc.dram_tensor("k3", [8, 128, 8, 512], FP16, kind="ExternalInput").ap()
    # [fp, p(h), hc, f-cols 256]
    m1_d = nc.dram_tensor("m1", [4, 128, 32, 256], FP16, kind="ExternalInput").ap()
    m2_d = nc.dram_tensor("m2", [4, 128, 32, 256], FP16, kind="ExternalInput").ap()
    m3_d = nc.dram_tensor("m3", [4, 128, 32, 256], FP16, kind="ExternalInput").ap()
    # [p(f), fc, 256] - cols 0:88 gate, 128:216 noise
    wgn_d = nc.dram_tensor("wgn", [128, 8, 256], F32R, kind="ExternalInput").ap()
    hm_d = nc.dram_tensor("hmrow", [1, 256], FP16, kind="ExternalInput").ap()
    alt_d = nc.dram_tensor("altrow", [1, 128], FP16, kind="ExternalInput").ap()
    b1_d = nc.dram_tensor("b1all", [128, 64], F32, kind="ExternalInput").ap()  # r|i
    b2_d = nc.dram_tensor("b2all", [128, 16], F32, kind="ExternalInput").ap()  # r|i
    eps_d = nc.dram_tensor("eps", [128, 2, E], F32, kind="ExternalInput").ap()
    out_d = nc.dram_tensor("out", [BL, E], F32, kind="ExternalOutput").ap()
    if _DEBUG_DUMP:
        dbg_het = nc.dram_tensor("dbg_het", [128, 8, 256], FP16, kind="ExternalOutput").ap()
        dbg_hot = nc.dram_tensor("dbg_hot", [128, 8, 256], FP16, kind="ExternalOutput").ap()
        dbg_xr = nc.dram_tensor("dbg_xr", [128, 8, 256], FP16, kind="ExternalOutput").ap()
        dbg_xi = nc.dram_tensor("dbg_xi", [128, 8, 256], FP16, kind="ExternalOutput").ap()
        dbg_o1r = nc.dram_tensor("dbg_o1r", [128, 32, 256], FP16, kind="ExternalOutput").ap()
        dbg_amp = nc.dram_tensor("dbg_amp", [128, 8, 256], F32R, kind="ExternalOutput").ap()

    with tile.TileContext(nc) as tc:
        with tc.tile_pool(name="consts", bufs=1) as consts, \
             tc.tile_pool(name="xstream", bufs=4) as xstream, \
             tc.tile_pool(name="wring", bufs=6) as wring, \
             tc.tile_pool(name="h8", bufs=1) as h8, \
             tc.tile_pool(name="acts", bufs=1) as acts, \
             tc.tile_pool(name="o16", bufs=1) as o16, \
             tc.tile_pool(name="stage", bufs=4) as stage, \
             tc.tile_pool(name="ps", bufs=1, space="PSUM") as ps:

            hm_sb = consts.tile([1, 256], FP16, tag="hm")
            nc.sync.dma_start(hm_sb, hm_d)
            alt_sb = consts.tile([1, 128], FP16, tag="alt")
            nc.sync.dma_start(alt_sb, alt_d)
            b1_sb = consts.tile([128, 64], F32, tag="b1")
            nc.sync.dma_start(b1_sb, b1_d)
            b2_sb = consts.tile([128, 16], F32, tag="b2")
            nc.sync.dma_start(b2_sb, b2_d)
            eps_sb = consts.tile([128, 2, E], F32, tag="eps")
            nc.sync.dma_start(eps_sb, eps_d)
            wgn_sb = consts.tile([128, 8, 256], F32R, tag="wgn")

            # DFT half-matrices on a dedicated tag (persist through stage A);
            # split into kc-halves so their DMA interleaves with the x stream
            cs_sb = {}
            for half in range(2):
                cs_sb[("c", half)] = wring.tile([128, 4, 8, 128], FP16,
                                                tag="cs", bufs=4,
                                                name=f"chs{half}")
                cs_sb[("s", half)] = wring.tile([128, 4, 8, 128], FP16,
                                                tag="cs", bufs=4,
                                                name=f"shs{half}")

            # persistent activations
            HeT = h8.tile([128, 8, 256], FP16, tag="he", name="HeT")
            HoT = h8.tile([128, 8, 256], FP16, tag="ho", name="HoT")
            xrT = acts.tile([128, 8, 256], FP16, tag="xr")
            xiT = acts.tile([128, 8, 256], FP16, tag="xi")
            # 3rd stream: xs = xr+xi (Karatsuba) or xin = -xi (plain)
            x3T = acts.tile([128, 8, 256], FP16, tag="x3")
            ampT = acts.tile([128, 8, 256], F32R, tag="amp")
            o1rT = o16.tile([128, 32, 256], FP16, tag="o1r", name="o1rT")
            o1iT = o16.tile([128, 32, 256], FP16, tag="o1i", name="o1iT")
            o1sT = None
            if KARA2:
                o1sT = o16.tile([128, 32, 256], FP16, tag="o1s", name="o1sT")

            # ---------------- Stage A + B ----------------
            scopeA = nc.named_scope("stageA_dft"); scopeA.__enter__()

            psB = {}

            def bbank(fc):
                # fc 7 borrows the gate-accumulator bank (idle until stage D)
                # so all 8 DFT f-chunks chase the x stream in one wave
                if fc == 7:
                    psB[fc] = ps.tile([128, 512], F32, tag="pg", bufs=1,
                                      name="B7")
                else:
                    psB[fc] = ps.tile([128, 512], F32, tag="bank", bufs=7,
                                      name=f"B{fc}")

            def b_mms(fc, lc):
                ch = cs_sb[("c", lc // 4)]
                sh = cs_sb[("s", lc // 4)]
                nc.tensor.matmul(psB[fc][:, 0:256], ch[:, lc % 4, fc],
                                 HeT[:, lc], start=(lc == 0), stop=False)
                nc.tensor.matmul(psB[fc][:, 256:512], sh[:, lc % 4, fc],
                                 HoT[:, lc], start=False, stop=(lc == 7))

            def b_mid(fc):
                nc.tensor.matmul(psB[fc][:, 0:256], alt_sb, hm_sb,
                                 start=False, stop=True)

            def b_evac(fc):
                nc.scalar.copy(xrT[:, fc], psB[fc][:, 0:256])
                nc.scalar.copy(xiT[:, fc], psB[fc][:, 256:512])
                if KARA1:
                    # one-PSUM-operand rule: xr is already in SBUF (fp16)
                    nc.vector.tensor_tensor(x3T[:, fc], xrT[:, fc],
                                            psB[fc][:, 256:512], op=ADD)
                else:
                    nc.vector.tensor_scalar(x3T[:, fc], psB[fc][:, 256:512],
                                            -1.0, None, op0=MULT)

            for fc in range(8):
                bbank(fc)
            for lc in range(8):
                for eo in range(2):
                    xa = xstream.tile([128, CH, 256], FP16, tag="big",
                                      name=f"x{eo}_{lc}")
                    nc.sync.dma_start(xa, xeo_d[eo][lc])
                    nc.vector.tensor_tensor(xa[:, 0:8], xa[:, 0:8],
                                            xa[:, 8:16], op=ADD)
                    nc.vector.tensor_tensor(xa[:, 0:4], xa[:, 0:4],
                                            xa[:, 4:8], op=ADD)
                    nc.vector.tensor_tensor(xa[:, 0:2], xa[:, 0:2],
                                            xa[:, 2:4], op=ADD)
                    dst = HeT if eo == 0 else HoT
                    nc.vector.tensor_tensor(dst[:, lc], xa[:, 0], xa[:, 1],
                                            op=ADD)
                # CS halves queue behind the first x chunks (so the x stream
                # gets the early bandwidth) but are emitted BEFORE the first
                # b_mms that read them — emission order defines dependencies
                if lc == 0:
                    nc.sync.dma_start(cs_sb[("c", 0)], chs_d[:, 0:4])
                    nc.sync.dma_start(cs_sb[("s", 0)], shs_d[:, 0:4])
                if lc == 2:
                    nc.sync.dma_start(cs_sb[("c", 1)], chs_d[:, 4:8])
                    nc.sync.dma_start(cs_sb[("s", 1)], shs_d[:, 4:8])
                for fc in range(8):
                    b_mms(fc, lc)
            for fc in range(8):
                b_mid(fc)
            for fc in range(8):
                b_evac(fc)

            if _DEBUG_DUMP:
                nc.sync.dma_start(dbg_het, HeT)
                nc.sync.dma_start(dbg_hot, HoT)
                nc.sync.dma_start(dbg_xr, xrT)
                nc.sync.dma_start(dbg_xi, xiT)

            scopeA.__exit__(None, None, None)
            scopeC = nc.named_scope("stageC_l1"); scopeC.__enter__()

            # L2 weight tiles, hc-half-split 1 MB each. DMAs are hoisted into
            # stage C's window (DMA is otherwise idle in late C while stage D
            # alone would need ~290 GB/s). fp0's first tiles ride the idle
            # xstream slots (same 8 KB/partition shape as the x chunks).
            nmat2 = 3 if KARA2 else 2
            mw = {}
            mds = (m1_d, m2_d, m3_d)[:nmat2]

            def m_tile(fp, mi, h, pool, tag):
                t = pool.tile([128, 16, 256], FP16, tag=tag,
                              name=f"m{mi}g{fp}h{h}")
                nc.sync.dma_start(t, mds[mi][fp][:, h * 16:(h + 1) * 16])
                mw[(fp, mi, h)] = t

            for hg in range(8):
                if hg == 3:
                    m_tile(0, 0, 0, xstream, "big")
                    m_tile(0, 1, 0, xstream, "big")
                    if KARA2:
                        m_tile(0, 2, 0, xstream, "big")
                    m_tile(0, 0, 1, xstream, "big")
                if hg == 5:
                    m_tile(0, 1, 1, wring, "w")
                    if KARA2:
                        m_tile(0, 2, 1, wring, "w")
                k1 = wring.tile([128, 8, 512], FP16, tag="w", name=f"k1g{hg}")
                nc.sync.dma_start(k1, k1_d[hg])
                k2 = wring.tile([128, 8, 512], FP16, tag="w", name=f"k2g{hg}")
                nc.sync.dma_start(k2, k2_d[hg])
                if hg == 7:
                    for mi in range(nmat2):
                        for h in range(2):
                            m_tile(1, mi, h, wring, "w")
                k3 = None
                if KARA1:
                    k3 = wring.tile([128, 8, 512], FP16, tag="w",
                                    name=f"k3g{hg}")
                    nc.sync.dma_start(k3, k3_d[hg])
                for j in range(4):
                    hc = hg * 4 + j
                    bA = ps.tile([128, 512], F32, tag="bank", bufs=7,
                                 name=f"cA{hc}")
                    bB = None
                    if KARA1:
                        bB = ps.tile([128, 512], F32, tag="bank", bufs=7,
                                     name=f"cB{hc}")
                    hsl = slice(j * 128, (j + 1) * 128)
                    for fc in range(8):
                        f0 = fc == 0
                        fl_ = fc == 7
                        if KARA1:
                            # m1 = (xr+xi)@W1r ; m2 = xi@(W1r+W1i)
                            # m3 = xr@(W1i-W1r)
                            nc.tensor.matmul(bA[:, 0:256], k1[:, fc, hsl],
                                             x3T[:, fc], start=f0, stop=fl_)
                            nc.tensor.matmul(bA[:, 256:512], k2[:, fc, hsl],
                                             xiT[:, fc], start=False, stop=fl_)
                            nc.tensor.matmul(bB[:, 0:256], k3[:, fc, hsl],
                                             xrT[:, fc], start=f0, stop=fl_)
                        else:
                            # o1r = xr@W1r + (-xi)@W1i ; o1i = xi@W1r + xr@W1i
                            nc.tensor.matmul(bA[:, 0:256], k1[:, fc, hsl],
                                             xrT[:, fc], start=f0, stop=False)
                            nc.tensor.matmul(bA[:, 0:256], k2[:, fc, hsl],
                                             x3T[:, fc], start=False, stop=fl_)
                            nc.tensor.matmul(bA[:, 256:512], k1[:, fc, hsl],
                                             xiT[:, fc], start=False, stop=False)
                            nc.tensor.matmul(bA[:, 256:512], k2[:, fc, hsl],
                                             xrT[:, fc], start=False, stop=fl_)
                    if KARA1:
                        # one-PSUM-operand rule: stage m1 through SBUF
                        tm = stage.tile([128, 256], F32, tag="d", bufs=4,
                                        name=f"tm_{hc}")
                        nc.scalar.copy(tm, bA[:, 0:256])
                        d1 = stage.tile([128, 256], F32, tag="d", bufs=4,
                                        name=f"d1_{hc}")
                        nc.vector.tensor_sub(d1, tm, bA[:, 256:512])
                        nc.scalar.activation(o1rT[:, hc], d1, AF.Relu,
                                             bias=b1_sb[:, hc:hc + 1])
                        d2 = stage.tile([128, 256], F32, tag="d", bufs=4,
                                        name=f"d2_{hc}")
                        nc.vector.tensor_add(d2, tm, bB[:, 0:256])
                        nc.scalar.activation(o1iT[:, hc], d2, AF.Relu,
                                             bias=b1_sb[:, 32 + hc:33 + hc])
                    else:
                        nc.scalar.activation(o1rT[:, hc], bA[:, 0:256], AF.Relu,
                                             bias=b1_sb[:, hc:hc + 1])
                        nc.scalar.activation(o1iT[:, hc], bA[:, 256:512],
                                             AF.Relu,
                                             bias=b1_sb[:, 32 + hc:33 + hc])
                    if KARA2:
                        nc.vector.tensor_tensor(o1sT[:, hc], o1rT[:, hc],
                                                o1iT[:, hc], op=ADD)

            scopeC.__exit__(None, None, None)
            scopeD = nc.named_scope("stageD_l2"); scopeD.__enter__()

            nc.sync.dma_start(wgn_sb, wgn_d)

            pgt = ps.tile([128, 2, 256], F32, tag="pg", bufs=1, name="pg")
            pg = [pgt[:, 0], pgt[:, 1]]

            for fp in range(4):
                # fp0/fp1 tiles were hoisted into stage C; stream fp+2 while
                # fp computes
                if fp + 2 <= 3:
                    for mi in range(nmat2):
                        for h in range(2):
                            m_tile(fp + 2, mi, h, wring, "w")
                for fl in range(2):
                    fc = fp * 2 + fl
                    bA = ps.tile([128, 512], F32, tag="bank", bufs=7,
                                 name=f"dA{fc}")
                    bB = ps.tile([128, 512], F32, tag="bank", bufs=7,
                                 name=f"dB{fc}")
                    fsl = slice(fl * 128, (fl + 1) * 128)
                    for hc in range(32):
                        h0 = hc == 0
                        hl = hc == 31
                        hh = hc // 16
                        hr = hc % 16
                        m1 = mw[(fp, 0, hh)]
                        m2 = mw[(fp, 1, hh)]
                        if KARA2:
                            m3 = mw[(fp, 2, hh)]
                            # m1 = o1s@W2r ; m2 = o1i@(W2r+W2i)
                            # m3 = o1r@(W2i-W2r)
                            nc.tensor.matmul(bA[:, 0:256], m1[:, hr, fsl],
                                             o1sT[:, hc], start=h0, stop=hl)
                            nc.tensor.matmul(bA[:, 256:512], m2[:, hr, fsl],
                                             o1iT[:, hc], start=False, stop=hl)
                            nc.tensor.matmul(bB[:, 0:256], m3[:, hr, fsl],
                                             o1rT[:, hc], start=h0, stop=hl)
                        else:
                            # qa|qb in bA, qi in bB:
                            # o2r = qa - qb ; o2i = qi
                            nc.tensor.matmul(bA[:, 0:256], m1[:, hr, fsl],
                                             o1rT[:, hc], start=h0, stop=hl)
                            nc.tensor.matmul(bA[:, 256:512], m2[:, hr, fsl],
                                             o1iT[:, hc], start=False, stop=hl)
                            nc.tensor.matmul(bB[:, 0:256], m1[:, hr, fsl],
                                             o1iT[:, hc], start=h0, stop=False)
                            nc.tensor.matmul(bB[:, 0:256], m2[:, hr, fsl],
                                             o1rT[:, hc], start=False, stop=hl)
                    # one-PSUM-operand rule: stage m1 through SBUF first
                    tm = stage.tile([128, 256], F32, tag="d", bufs=4,
                                    name=f"tm{fc}")
                    nc.scalar.copy(tm, bA[:, 0:256])
                    sd = stage.tile([128, 256], F32, tag="d", bufs=4,
                                    name=f"sd{fc}")
                    nc.vector.tensor_sub(sd, tm, bA[:, 256:512])
                    t1 = stage.tile([128, 256], F32, tag="d", bufs=4,
                                    name=f"sqr{fc}")
                    nc.scalar.activation(t1, sd, AF.Square,
                                         bias=b2_sb[:, fc:fc + 1])
                    t2 = stage.tile([128, 256], F32, tag="d", bufs=4,
                                    name=f"sqi{fc}")
                    if KARA2:
                        si = stage.tile([128, 256], F32, tag="d", bufs=4,
                                        name=f"si{fc}")
                        nc.vector.tensor_add(si, tm, bB[:, 0:256])
                        nc.scalar.activation(t2, si, AF.Square,
                                             bias=b2_sb[:, 8 + fc:9 + fc])
                    else:
                        nc.scalar.activation(t2, bB[:, 0:256], AF.Square,
                                             bias=b2_sb[:, 8 + fc:9 + fc])
                    nc.vector.tensor_add(t1, t1, t2)
                    nc.scalar.activation(ampT[:, fc], t1, AF.Sqrt)
                    for bt in range(2):
                        bs = slice(bt * 128, (bt + 1) * 128)
                        # pg[0]/pg[1] share one PSUM bank: only the very first
                        # matmul may set start (a start wipes the whole bank)
                        nc.tensor.matmul(pg[bt], ampT[:, fc, bs], wgn_sb[:, fc],
                                         start=(fc == 0 and bt == 0),
                                         stop=(fc == 7))

            if _DEBUG_DUMP:
                nc.sync.dma_start(dbg_o1r, o1rT)
                nc.sync.dma_start(dbg_amp, ampT)

            scopeD.__exit__(None, None, None)
            scopeE = nc.named_scope("stageE_gate"); scopeE.__enter__()

            out_v = out_d.rearrange("(bt p) e -> bt p e", bt=2)
            # softplus/noise chain packed over both bt halves (fewer
            # instructions + activation-table switches on the serial tail)
            logits2 = stage.tile([128, 2, E], F32, tag="logits", bufs=1)
            if training:
                stdn2 = stage.tile([128, 2, E], F32, tag="stdn", bufs=1)
                # softplus(z) = ln(1 + exp(z))
                nc.scalar.activation(stdn2, pgt[:, :, 128:128 + E], AF.Exp)
                nc.vector.tensor_scalar_add(stdn2, stdn2, 1.0)
                nc.scalar.activation(stdn2, stdn2, AF.Ln)
                # (softplus(z)+eps0)*eps
                nc.vector.scalar_tensor_tensor(
                    stdn2, stdn2, float(NOISE_EPS), eps_sb,
                    op0=ADD, op1=MULT)
                nc.vector.tensor_tensor(logits2, pgt[:, :, 0:E], stdn2,
                                        op=ADD)
            else:
                nc.vector.tensor_copy(logits2, pgt[:, :, 0:E])
            for bt in range(2):
                logits = logits2[:, bt]
                top8 = stage.tile([128, 8], F32, tag="top8", bufs=2)
                nc.vector.max(top8, logits)
                negmax = stage.tile([128, 1], F32, tag="negmax", bufs=2)
                nc.vector.tensor_scalar(negmax, top8[:, 0:1], -1.0, None,
                                        op0=MULT)
                ex = stage.tile([128, E], F32, tag="ex", bufs=2)
                nc.scalar.activation(ex, logits, AF.Exp, bias=negmax)
                msk = stage.tile([128, E], F32, tag="msk", bufs=2)
                nc.vector.tensor_scalar(msk, logits, top8[:, 2:3], None,
                                        op0=mybir.AluOpType.is_ge)
                nc.vector.tensor_mul(ex, ex, msk)
                ssum = stage.tile([128, 1], F32, tag="ssum", bufs=2)
                nc.vector.reduce_sum(out=ssum, in_=ex, axis=mybir.AxisListType.X)
                rinv = stage.tile([128, 1], F32, tag="rinv", bufs=2)
                nc.vector.reciprocal(rinv, ssum)
                gates = stage.tile([128, E], F32, tag="gates", bufs=2)
                nc.vector.tensor_scalar(gates, ex, rinv, None, op0=MULT)
                nc.sync.dma_start(out_v[bt], gates)

            scopeE.__exit__(None, None, None)

    nc.compile()
    return nc


_PROGRAM_CACHE = {}


def _get_program(training: bool):
    key = bool(training)
    if key not in _PROGRAM_CACHE:
        _PROGRAM_CACHE[key] = _build_program(key)
    return _PROGRAM_CACHE[key]


def _prep_inputs(x, fc_w, fc_b, w1, b1, w2, b2, w_gate, w_noise, eps):
    f32 = np.float32
    f16 = np.float16

    # ---- weights/constants shared by all cores ----
    ll = np.arange(1, F, dtype=np.int64)[:, None]  # l' = 1..1023
    ff = np.arange(1, F + 1, dtype=np.int64)[None, :]
    ang = 2.0 * np.pi * ((ll * ff) % L).astype(np.float64) / L
    scale = 1.0 / np.sqrt(L)
    Ch = np.empty((F, F), np.float64)
    Sh = np.empty((F, F), np.float64)
    Ch[0, :] = scale
    Ch[1:, :] = np.cos(ang) * scale
    Sh[0, :] = 0.0
    Sh[1:, :] = -np.sin(ang) * scale
    # [p(l'), kc, fc, 128]
    chs = Ch.astype(f16).reshape(8, 128, 8, 128).transpose(1, 0, 2, 3)
    shs = Sh.astype(f16).reshape(8, 128, 8, 128).transpose(1, 0, 2, 3)

    # midpoint row: C[1024, f] = (-1)^f * scale, f = p+1 within each chunk
    p = np.arange(128)
    alt = (np.where((p + 1) % 2 == 0, 1.0, -1.0) * scale).astype(f16)
    alt = alt.reshape(1, 128)

    w1r = np.asarray(w1[0], f32)
    w1i = np.asarray(w1[1], f32)
    if KARA1:
        k1m, k2m, k3m = w1r, w1r + w1i, w1i - w1r
    else:
        k1m, k2m, k3m = w1r, w1i, w1i  # k3 unused

    def tile1(M):
        return np.ascontiguousarray(
            M.reshape(8, 128, 8, 512).transpose(2, 1, 0, 3).astype(f16))

    w2r = np.asarray(w2[0], f32)
    w2i = np.asarray(w2[1], f32)
    if KARA2:
        m1m, m2m, m3m = w2r, w2r + w2i, w2i - w2r
    else:
        m1m, m2m, m3m = w2r, w2i, w2i  # m3 unused

    def tile2(M):
        return np.ascontiguousarray(
            M.reshape(32, 128, 4, 256).transpose(2, 1, 0, 3).astype(f16))

    wgn = np.zeros((F, 256), f32)
    wgn[:, 0:E] = np.asarray(w_gate, f32)
    wgn[:, 128:128 + E] = np.asarray(w_noise, f32)
    wgn = rnd11(wgn).reshape(8, 128, 256).transpose(1, 0, 2)

    b1all = np.zeros((128, 64), f32)
    b1all[:, 0:32] = np.asarray(b1[0], f32).reshape(32, 128).T
    b1all[:, 32:64] = np.asarray(b1[1], f32).reshape(32, 128).T
    b2all = np.zeros((128, 16), f32)
    b2all[:, 0:8] = np.asarray(b2[0], f32).reshape(8, 128).T
    b2all[:, 8:16] = np.asarray(b2[1], f32).reshape(8, 128).T

    common = {
        "chs": np.ascontiguousarray(chs),
        "shs": np.ascontiguousarray(shs),
        "k1": tile1(k1m),
        "k2": tile1(k2m),
        "k3": tile1(k3m),
        "m1": tile2(m1m),
        "m2": tile2(m2m),
        "m3": tile2(m3m),
        "wgn": np.ascontiguousarray(wgn),
        "altrow": alt,
        "b1all": b1all,
        "b2all": b2all,
    }

    # ---- per-core data ----
    x = np.asarray(x, f32)
    fcw = np.asarray(fc_w, f32).reshape(CH)
    eps = np.asarray(eps, f32)

    in_maps = []
    for i in range(NCORES):
        xs = x[i * BL:(i + 1) * BL]  # [256, 2048, 16]
        # xw[b, c, l] = x[b, l, c] * fc_w[c] (scale folded host-side)
        xw = xs.transpose(0, 2, 1) * fcw[None, :, None]  # [256, 16, 2048]
        xe = np.empty((BL, CH, F), f32)
        xo = np.empty((BL, CH, F), f32)
        xe[:, :, 0] = xw[:, :, 0]
        xo[:, :, 0] = 0.0
        fwd = xw[:, :, 1:1024]
        rev = xw[:, :, 2047:1024:-1]
        xe[:, :, 1:1024] = fwd + rev
        xo[:, :, 1:1024] = fwd - rev
        hm = xw[:, :, 1024].sum(axis=1)  # [256]
        # tile: [eo, b, c, l'] -> [eo, lc, p(l'), c, b]
        xeo = np.stack([xe, xo])  # [2, 256, 16, 1024]
        xeo = xeo.transpose(0, 3, 2, 1).astype(f16)  # [2, 1024, 16, 256]
        sh = dict(common)
        sh["xeo"] = np.ascontiguousarray(xeo.reshape(2, 8, 128, CH, 256))
        sh["hmrow"] = hm.astype(f16).reshape(1, 256)
        esh = eps[i * BL:(i + 1) * BL]  # [256, E]
        sh["eps"] = np.ascontiguousarray(esh.reshape(2, 128, E).transpose(1, 0, 2))
        in_maps.append(sh)
    return in_maps


def run(inputs, trace=False):
    """Returns (gates [B, E] float32, BassKernelResults)."""
    x = np.asarray(inputs["x"], np.float32)
    fc_w = np.asarray(inputs["fc_w"], np.float32)
    fc_b = np.asarray(inputs["fc_b"], np.float32)
    w1 = np.asarray(inputs["w1"], np.float32)
    b1 = np.asarray(inputs["b1"], np.float32)
    w2 = np.asarray(inputs["w2"], np.float32)
    b2 = np.asarray(inputs["b2"], np.float32)
    w_gate = np.asarray(inputs["w_gate"], np.float32)
    w_noise = np.asarray(inputs["w_noise"], np.float32)
    eps = np.asarray(inputs["eps"], np.float32)
    training = bool(int(np.asarray(inputs.get("training", 1))))

    nc = _get_program(training)
    in_maps = _prep_inputs(x, fc_w, fc_b, w1, b1, w2, b2, w_gate, w_noise, eps)
    res = run_bass_kernel_spmd(
        nc, in_maps, core_ids=list(range(NCORES)), trace=trace,
    )
    gates = np.concatenate([r["out"] for r in res.results], axis=0)
    return gates.astype(np.float32), res


def kernel(**inputs):
    gates, _ = run(inputs, trace=False)
    return gates


# revision 35
# speedup vs baseline: 1.0623x; 1.0623x over previous
"""Trainium2 Bass kernel v3 for nn_AdaptiveFourierTransformGateLayer.

Data-parallel over batch: 8 cores x 256 rows. Per core:

  Host prep: xw = x * fc_w (scale+layout only), reflection-fold over l:
    xe[b,c,l'] = xw[b,l',c] + xw[b,2048-l',c]   (l'=1..1023; l'=0 -> xw[b,0,c])
    xo[b,c,l'] = xw[b,l',c] - xw[b,2048-l',c]   (l'=0 -> 0)
    hm[b] = sum_c xw[b,1024,c]                  (midpoint row)
  Folding halves the DFT to 1024x1024 half-matrices (C even / S odd).
  fc_b is dropped: AC-bin column sums of the DFT are exactly zero.
  Everything streamed in fp16, laid out [l'-part, c, b] so the channel
  tree-reduce lands directly in DFT-ready [l', b] layout (no transposes).

  Device:
  A: c-tree reduction (DVE, fp16 2x mode) -> HeT/HoT [l'-part, b].
  B: DFT chase: per f-chunk PSUM bank holds xr | xi halves; fp16 matmuls
     Ch-chunk^T @ HeT / Sh-chunk^T @ HoT accumulate as l'-chunks arrive.
     Midpoint rank-1 term alt(f) x hm(b) closes xr. 7 banks chase, f-chunk
     7 runs as a second wave after bank 0 evacuates. Evac to fp16
     xr/xi/xs (xs = xr+xi for Karatsuba).
  C: layer 1 via 3-matmul Karatsuba complex product:
       m1 = (xr+xi)@W1r, m2 = xi@(W1r+W1i), m3 = xr@(W1i-W1r)
       o1r = relu(m1-m2+b1r), o1i = relu(m1+m3+b1i), o1s = o1r+o1i
     m1|m2 share a PSUM bank, m3 in a second bank. Transposed dataflow
     (stationary = weight chunk, moving = activations [128,256]).
  D: layer 2 same Karatsuba shape; amp = sqrt((m1-m2+b2r)^2+(m1+m3+b2i)^2)
     -> ampT f32r. Gate matmuls (ampT @ wgn, f32r) chased per f-chunk.
  E: noisy top-3 softmax -> gates (small DVE/Act chain only).
"""
import sys
import types
import contextlib
import ctypes

import numpy as np

if "/opt/trn_rl_repo" not in sys.path:
    sys.path.insert(0, "/opt/trn_rl_repo")

# ---------------------------------------------------------------------------
# NTFF trace hook shim (only used when trace=True; harmless otherwise)
# ---------------------------------------------------------------------------


def _install_trace_shim():
    if "antenv.axon_hooks" in sys.modules:
        return
    so_path = "/opt/axon/libaxon_pjrt.so"

    def _mk():
        try:
            lib = ctypes.CDLL(so_path)
        except OSError:
            return None
        if not hasattr(lib, "axon_start_nrt_profile"):
            return None
        lib.axon_start_nrt_profile.argtypes = [
            ctypes.POINTER(ctypes.c_int64),
            ctypes.c_size_t,
        ]
        lib.axon_start_nrt_profile.restype = ctypes.c_int64
        lib.axon_stop_nrt_profile.argtypes = [ctypes.c_char_p]
        lib.axon_stop_nrt_profile.restype = ctypes.c_int64

        @contextlib.contextmanager
        def _hook(output_dir, device_ids):
            import jax

            jax.devices()
            if device_ids:
                ids = (ctypes.c_int64 * len(device_ids))(*device_ids)
                rc = lib.axon_start_nrt_profile(ids, len(device_ids))
            else:
                rc = lib.axon_start_nrt_profile(None, 0)
            if rc != 0:
                raise RuntimeError(f"axon_start_nrt_profile rc={rc}")
            try:
                yield
            finally:
                n = lib.axon_stop_nrt_profile(str(output_dir).encode())
                print(f"profile: {n} file(s) written to {output_dir}", file=sys.stderr)

        return _hook

    mod = types.ModuleType("antenv.axon_hooks")
    mod._hook = _mk()
    mod.get_axon_ntff_profile_hook = lambda: mod._hook
    mod.set_axon_ntff_profile_hook = lambda h: setattr(mod, "_hook", h)
    sys.modules["antenv.axon_hooks"] = mod
    try:
        import antenv

        antenv.axon_hooks = mod
    except ImportError:
        pass


_install_trace_shim()

import concourse.tile as tile  # noqa: E402
from concourse import bacc, mybir  # noqa: E402
from concourse.bass_utils import run_bass_kernel_spmd  # noqa: E402

# ---------------------------------------------------------------------------
# Problem constants (hardcoded)
# ---------------------------------------------------------------------------
B = 2048
L = 2048
CH = 16
F = 1024  # num freqs (rfft bins 1..1024)
FH = 4096  # hidden
E = 88  # num experts
NOISE_EPS = 0.01
_DEBUG_DUMP = False
NCORES = 8
BL = B // NCORES  # 256 rows per core
F32R = mybir.dt.float32r
F32 = mybir.dt.float32
FP16 = mybir.dt.float16

KARA1 = True  # Karatsuba in layer 1 (exact fp16 combos; revert to False
              # if the flip count pushes rel err near 2e-2)
KARA2 = True  # ... in layer 2

ADD = mybir.AluOpType.add
MULT = mybir.AluOpType.mult
AF = mybir.ActivationFunctionType


def rnd11(x):
    """Round-to-nearest keeping 11 mantissa bits (hardware f32r rounding)."""
    a = np.ascontiguousarray(x, np.float32)
    ai = a.view(np.uint32)
    return ((ai + np.uint32(1 << 11)) & np.uint32(0xFFFFF000)).view(np.float32)


def _build_program(training: bool):
    nc = bacc.Bacc("TRN2", target_bir_lowering=False, debug=False, num_devices=NCORES)

    # [eo, lc, p(l'), c, b] - host pre-transposed so tree-reduce -> [l', b]
    xeo_d = nc.dram_tensor("xeo", [2, 8, 128, CH, 256], FP16,
                           kind="ExternalInput").ap()
    # [p(l'), kc, fc, 128 f-cols]
    chs_d = nc.dram_tensor("chs", [128, 8, 8, 128], FP16, kind="ExternalInput").ap()
    shs_d = nc.dram_tensor("shs", [128, 8, 8, 128], FP16, kind="ExternalInput").ap()
    # [hg, p(f), fc, h-cols 512]
    k1_d = nc.dram_tensor("k1", [8, 128, 8, 512], FP16, kind="ExternalInput").ap()
    k2_d = nc.dram_tensor("k2", [8, 128, 8, 512], FP16, kind="ExternalInput").ap()
    k3_d = nc.dram_tensor("k3", [8, 128, 8, 512], FP16, kind="ExternalInput").ap()
    # [fp, p(h), hc, f-cols 256]
    m1_d = nc.dram_tensor("m1", [4, 128, 32, 256], FP16, kind="ExternalInput").ap()
    m2_d = nc.dram_tensor("m2", [4, 128, 32, 256], FP16, kind="ExternalInput").ap()
    m3_d = nc.dram_tensor("m3", [4, 128, 32, 256], FP16, kind="ExternalInput").ap()
    # [p(f), fc, 256] - cols 0:88 gate, 128:216 noise
    wgn_d = nc.dram_tensor("wgn", [128, 8, 256], F32R, kind="ExternalInput").ap()
    hm_d = nc.dram_tensor("hmrow", [1, 256], FP16, kind="ExternalInput").ap()
    alt_d = nc.dram_tensor("altrow", [1, 128], FP16, kind="ExternalInput").ap()
    b1_d = nc.dram_tensor("b1all", [128, 64], F32, kind="ExternalInput").ap()  # r|i
    b2_d = nc.dram_tensor("b2all", [128, 16], F32, kind="ExternalInput").ap()  # r|i
    eps_d = nc.dram_tensor("eps", [128, 2, E], F32, kind="ExternalInput").ap()
    out_d = nc.dram_tensor("out", [BL, E], F32, kind="ExternalOutput").ap()
    if _DEBUG_DUMP:
        dbg_het = nc.dram_tensor("dbg_het", [128, 8, 256], FP16, kind="ExternalOutput").ap()
        dbg_hot = nc.dram_tensor("dbg_hot", [128, 8, 256], FP16, kind="ExternalOutput").ap()
        dbg_xr = nc.dram_tensor("dbg_xr", [128, 8, 256], FP16, kind="ExternalOutput").ap()
        dbg_xi = nc.dram_tensor("dbg_xi", [128, 8, 256], FP16, kind="ExternalOutput").ap()
        dbg_o1r = nc.dram_tensor("dbg_o1r", [128, 32, 256], FP16, kind="ExternalOutput").ap()
        dbg_amp = nc.dram_tensor("dbg_amp", [128, 8, 256], F32R, kind="ExternalOutput").ap()

    with tile.TileContext(nc) as tc:
        with tc.tile_pool(name="consts", bufs=1) as consts, \
             tc.tile_pool(name="xstream", bufs=4) as xstream, \
             tc.tile_pool(name="wring", bufs=6) as wring, \
             tc.tile_pool(name="h8", bufs=1) as h8, \
             tc.tile_pool(name="acts", bufs=1) as acts, \
             tc.tile_pool(name="o16", bufs=1) as o16, \
             tc.tile_pool(name="stage", bufs=4) as stage, \
             tc.tile_pool(name="ps", bufs=1, space="PSUM") as ps:

            hm_sb = consts.tile([1, 256], FP16, tag="hm")
            nc.sync.dma_start(hm_sb, hm_d)
            alt_sb = consts.tile([1, 128], FP16, tag="alt")
            nc.sync.dma_start(alt_sb, alt_d)
            b1_sb = consts.tile([128, 64], F32, tag="b1")
            nc.sync.dma_start(b1_sb, b1_d)
            b2_sb = consts.tile([128, 16], F32, tag="b2")
            nc.sync.dma_start(b2_sb, b2_d)
            eps_sb = consts.tile([128, 2, E], F32, tag="eps")
            nc.sync.dma_start(eps_sb, eps_d)
            wgn_sb = consts.tile([128, 8, 256], F32R, tag="wgn")

            # DFT half-matrices on a dedicated tag (persist through stage A);
            # split into kc-halves so their DMA interleaves with the x stream
            cs_sb = {}
            for half in range(2):
                cs_sb[("c", half)] = wring.tile([128, 4, 8, 128], FP16,
                                                tag="cs", bufs=4,
                                                name=f"chs{half}")
                cs_sb[("s", half)] = wring.tile([128, 4, 8, 128], FP16,
                                                tag="cs", bufs=4,
                                                name=f"shs{half}")

            # persistent activations
            HeT = h8.tile([128, 8, 256], FP16, tag="he", name="HeT")
            HoT = h8.tile([128, 8, 256], FP16, tag="ho", name="HoT")
            xrT = acts.tile([128, 8, 256], FP16, tag="xr")
            xiT = acts.tile([128, 8, 256], FP16, tag="xi")
            # 3rd stream: xs = xr+xi (Karatsuba) or xin = -xi (plain)
            x3T = acts.tile([128, 8, 256], FP16, tag="x3")
            ampT = acts.tile([128, 8, 256], F32R, tag="amp")
            o1rT = o16.tile([128, 32, 256], FP16, tag="o1r", name="o1rT")
            o1iT = o16.tile([128, 32, 256], FP16, tag="o1i", name="o1iT")
            o1sT = None
            if KARA2:
                o1sT = o16.tile([128, 32, 256], FP16, tag="o1s", name="o1sT")

            # ---------------- Stage A + B ----------------
            scopeA = nc.named_scope("stageA_dft"); scopeA.__enter__()

            psB = {}

            def bbank(fc):
                # fc 7 borrows the gate-accumulator bank (idle until stage D)
                # so all 8 DFT f-chunks chase the x stream in one wave
                if fc == 7:
                    psB[fc] = ps.tile([128, 512], F32, tag="pg", bufs=1,
                                      name="B7")
                else:
                    psB[fc] = ps.tile([128, 512], F32, tag="bank", bufs=7,
                                      name=f"B{fc}")

            def b_mms(fc, lc):
                ch = cs_sb[("c", lc // 4)]
                sh = cs_sb[("s", lc // 4)]
                nc.tensor.matmul(psB[fc][:, 0:256], ch[:, lc % 4, fc],
                                 HeT[:, lc], start=(lc == 0), stop=False)
                nc.tensor.matmul(psB[fc][:, 256:512], sh[:, lc % 4, fc],
                                 HoT[:, lc], start=False, stop=(lc == 7))

            def b_mid(fc):
                nc.tensor.matmul(psB[fc][:, 0:256], alt_sb, hm_sb,
                                 start=False, stop=True)

            def b_evac(fc):
                nc.scalar.copy(xrT[:, fc], psB[fc][:, 0:256])
                nc.scalar.copy(xiT[:, fc], psB[fc][:, 256:512])
                if KARA1:
                    # one-PSUM-operand rule: xr is already in SBUF (fp16)
                    nc.vector.tensor_tensor(x3T[:, fc], xrT[:, fc],
                                            psB[fc][:, 256:512], op=ADD)
                else:
                    nc.vector.tensor_scalar(x3T[:, fc], psB[fc][:, 256:512],
                                            -1.0, None, op0=MULT)

            for fc in range(8):
                bbank(fc)
            for lc in range(8):
                for eo in range(2):
                    xa = xstream.tile([128, CH, 256], FP16, tag="big",
                                      name=f"x{eo}_{lc}")
                    nc.sync.dma_start(xa, xeo_d[eo][lc])
                    nc.vector.tensor_tensor(xa[:, 0:8], xa[:, 0:8],
                                            xa[:, 8:16], op=ADD)
                    nc.vector.tensor_tensor(xa[:, 0:4], xa[:, 0:4],
                                            xa[:, 4:8], op=ADD)
                    nc.vector.tensor_tensor(xa[:, 0:2], xa[:, 0:2],
                                            xa[:, 2:4], op=ADD)
                    dst = HeT if eo == 0 else HoT
                    nc.vector.tensor_tensor(dst[:, lc], xa[:, 0], xa[:, 1],
                                            op=ADD)
                # CS halves queue behind the first x chunks (so the x stream
                # gets the early bandwidth) but are emitted BEFORE the first
                # b_mms that read them — emission order defines dependencies
                if lc == 0:
                    nc.sync.dma_start(cs_sb[("c", 0)], chs_d[:, 0:4])
                    nc.sync.dma_start(cs_sb[("s", 0)], shs_d[:, 0:4])
                if lc == 2:
                    nc.sync.dma_start(cs_sb[("c", 1)], chs_d[:, 4:8])
                    nc.sync.dma_start(cs_sb[("s", 1)], shs_d[:, 4:8])
                for fc in range(8):
                    b_mms(fc, lc)
            for fc in range(8):
                b_mid(fc)
            for fc in range(8):
                b_evac(fc)

            if _DEBUG_DUMP:
                nc.sync.dma_start(dbg_het, HeT)
                nc.sync.dma_start(dbg_hot, HoT)
                nc.sync.dma_start(dbg_xr, xrT)
                nc.sync.dma_start(dbg_xi, xiT)

            scopeA.__exit__(None, None, None)
            scopeC = nc.named_scope("stageC_l1"); scopeC.__enter__()

            # L2 weight tiles, hc-half-split 1 MB each. DMAs are hoisted into
            # stage C's window (DMA is otherwise idle in late C while stage D
            # alone would need ~290 GB/s). fp0's first tiles ride the idle
            # xstream slots (same 8 KB/partition shape as the x chunks).
            nmat2 = 3 if KARA2 else 2
            mw = {}
            mds = (m1_d, m2_d, m3_d)[:nmat2]

            def m_tile(fp, mi, h, pool, tag):
                t = pool.tile([128, 16, 256], FP16, tag=tag,
                              bufs=(4 if tag in ("cs", "big") else 6),
                              name=f"m{mi}g{fp}h{h}")
                nc.sync.dma_start(t, mds[mi][fp][:, h * 16:(h + 1) * 16])
                mw[(fp, mi, h)] = t

            for hg in range(8):
                if hg == 5:
                    m_tile(0, 0, 0, xstream, "big")
                    m_tile(0, 1, 0, xstream, "big")
                    if KARA2:
                        m_tile(0, 2, 0, xstream, "big")
                    m_tile(0, 0, 1, xstream, "big")
                if hg == 6:
                    # ride the dead DFT-matrix slots (same 8 KB/partition);
                    # the w ring must stay k-only here or k3g6+ slot-waits a
                    # stage-D-read tile -> scheduling deadlock
                    m_tile(0, 1, 1, wring, "cs")
                    if KARA2:
                        m_tile(0, 2, 1, wring, "cs")
                k1 = wring.tile([128, 8, 512], FP16, tag="w", name=f"k1g{hg}")
                nc.sync.dma_start(k1, k1_d[hg])
                k2 = wring.tile([128, 8, 512], FP16, tag="w", name=f"k2g{hg}")
                nc.sync.dma_start(k2, k2_d[hg])
                k3 = None
                if KARA1:
                    k3 = wring.tile([128, 8, 512], FP16, tag="w",
                                    name=f"k3g{hg}")
                    nc.sync.dma_start(k3, k3_d[hg])
                if hg == 7:
                    # after ALL k DMAs: a k DMA queued behind these m tiles
                    # would deadlock (their ring slots free only once the
                    # last k tiles are consumed)
                    for mi in range(nmat2):
                        for h in range(2):
                            m_tile(1, mi, h, wring, "w")
                for j in range(4):
                    hc = hg * 4 + j
                    bA = ps.tile([128, 512], F32, tag="bank", bufs=7,
                                 name=f"cA{hc}")
                    bB = None
                    if KARA1:
                        bB = ps.tile([128, 512], F32, tag="bank", bufs=7,
                                     name=f"cB{hc}")
                    hsl = slice(j * 128, (j + 1) * 128)
                    for fc in range(8):
                        f0 = fc == 0
                        fl_ = fc == 7
                        if KARA1:
                            # m1 = (xr+xi)@W1r ; m2 = xi@(W1r+W1i)
                            # m3 = xr@(W1i-W1r)
                            nc.tensor.matmul(bA[:, 0:256], k1[:, fc, hsl],
                                             x3T[:, fc], start=f0, stop=fl_)
                            nc.tensor.matmul(bA[:, 256:512], k2[:, fc, hsl],
                                             xiT[:, fc], start=False, stop=fl_)
                            nc.tensor.matmul(bB[:, 0:256], k3[:, fc, hsl],
                                             xrT[:, fc], start=f0, stop=fl_)
                        else:
                            # o1r = xr@W1r + (-xi)@W1i ; o1i = xi@W1r + xr@W1i
                            nc.tensor.matmul(bA[:, 0:256], k1[:, fc, hsl],
                                             xrT[:, fc], start=f0, stop=False)
                            nc.tensor.matmul(bA[:, 0:256], k2[:, fc, hsl],
                                             x3T[:, fc], start=False, stop=fl_)
                            nc.tensor.matmul(bA[:, 256:512], k1[:, fc, hsl],
                                             xiT[:, fc], start=False, stop=False)
                            nc.tensor.matmul(bA[:, 256:512], k2[:, fc, hsl],
                                             xrT[:, fc], start=False, stop=fl_)
                    if KARA1:
                        # one-PSUM-operand rule: stage m1 through SBUF
                        tm = stage.tile([128, 256], F32, tag="d", bufs=4,
                                        name=f"tm_{hc}")
                        nc.scalar.copy(tm, bA[:, 0:256])
                        d1 = stage.tile([128, 256], F32, tag="d", bufs=4,
                                        name=f"d1_{hc}")
                        nc.vector.tensor_sub(d1, tm, bA[:, 256:512])
                        nc.scalar.activation(o1rT[:, hc], d1, AF.Relu,
                                             bias=b1_sb[:, hc:hc + 1])
                        d2 = stage.tile([128, 256], F32, tag="d", bufs=4,
                                        name=f"d2_{hc}")
                        nc.vector.tensor_add(d2, tm, bB[:, 0:256])
                        nc.scalar.activation(o1iT[:, hc], d2, AF.Relu,
                                             bias=b1_sb[:, 32 + hc:33 + hc])
                    else:
                        nc.scalar.activation(o1rT[:, hc], bA[:, 0:256], AF.Relu,
                                             bias=b1_sb[:, hc:hc + 1])
                        nc.scalar.activation(o1iT[:, hc], bA[:, 256:512],
                                             AF.Relu,
                                             bias=b1_sb[:, 32 + hc:33 + hc])
                    if KARA2:
                        nc.vector.tensor_tensor(o1sT[:, hc], o1rT[:, hc],
                                                o1iT[:, hc], op=ADD)

            scopeC.__exit__(None, None, None)
            scopeD = nc.named_scope("stageD_l2"); scopeD.__enter__()

            nc.sync.dma_start(wgn_sb, wgn_d)

            pgt = ps.tile([128, 2, 256], F32, tag="pg", bufs=1, name="pg")
            pg = [pgt[:, 0], pgt[:, 1]]

            for fp in range(4):
                # fp0/fp1 tiles were hoisted into stage C; stream fp+2 while
                # fp computes. fp+2's first tiles ride the big ring, whose
                # slots free as fp's hoisted tiles are consumed.
                if fp + 2 <= 3:
                    m_tile(fp + 2, 0, 0, xstream, "big")
                    m_tile(fp + 2, 1, 0, xstream, "big")
                    if KARA2:
                        m_tile(fp + 2, 2, 0, xstream, "big")
                    m_tile(fp + 2, 0, 1, xstream, "big")
                    m_tile(fp + 2, 1, 1, wring, "w")
                    if KARA2:
                        m_tile(fp + 2, 2, 1, wring, "w")
                for fl in range(2):
                    fc = fp * 2 + fl
                    bA = ps.tile([128, 512], F32, tag="bank", bufs=7,
                                 name=f"dA{fc}")
                    bB = ps.tile([128, 512], F32, tag="bank", bufs=7,
                                 name=f"dB{fc}")
                    fsl = slice(fl * 128, (fl + 1) * 128)
                    for hc in range(32):
                        h0 = hc == 0
                        hl = hc == 31
                        hh = hc // 16
                        hr = hc % 16
                        m1 = mw[(fp, 0, hh)]
                        m2 = mw[(fp, 1, hh)]
                        if KARA2:
                            m3 = mw[(fp, 2, hh)]
                            # m1 = o1s@W2r ; m2 = o1i@(W2r+W2i)
                            # m3 = o1r@(W2i-W2r)
                            nc.tensor.matmul(bA[:, 0:256], m1[:, hr, fsl],
                                             o1sT[:, hc], start=h0, stop=hl)
                            nc.tensor.matmul(bA[:, 256:512], m2[:, hr, fsl],
                                             o1iT[:, hc], start=False, stop=hl)
                            nc.tensor.matmul(bB[:, 0:256], m3[:, hr, fsl],
                                             o1rT[:, hc], start=h0, stop=hl)
                        else:
                            # qa|qb in bA, qi in bB:
                            # o2r = qa - qb ; o2i = qi
                            nc.tensor.matmul(bA[:, 0:256], m1[:, hr, fsl],
                                             o1rT[:, hc], start=h0, stop=hl)
                            nc.tensor.matmul(bA[:, 256:512], m2[:, hr, fsl],
                                             o1iT[:, hc], start=False, stop=hl)
                            nc.tensor.matmul(bB[:, 0:256], m1[:, hr, fsl],
                                             o1iT[:, hc], start=h0, stop=False)
                            nc.tensor.matmul(bB[:, 0:256], m2[:, hr, fsl],
                                             o1rT[:, hc], start=False, stop=hl)
                    # one-PSUM-operand rule: stage m1 through SBUF first
                    tm = stage.tile([128, 256], F32, tag="d", bufs=4,
                                    name=f"tm{fc}")
                    nc.scalar.copy(tm, bA[:, 0:256])
                    sd = stage.tile([128, 256], F32, tag="d", bufs=4,
                                    name=f"sd{fc}")
                    nc.vector.tensor_sub(sd, tm, bA[:, 256:512])
                    t1 = stage.tile([128, 256], F32, tag="d", bufs=4,
                                    name=f"sqr{fc}")
                    nc.scalar.activation(t1, sd, AF.Square,
                                         bias=b2_sb[:, fc:fc + 1])
                    t2 = stage.tile([128, 256], F32, tag="d", bufs=4,
                                    name=f"sqi{fc}")
                    if KARA2:
                        si = stage.tile([128, 256], F32, tag="d", bufs=4,
                                        name=f"si{fc}")
                        nc.vector.tensor_add(si, tm, bB[:, 0:256])
                        nc.scalar.activation(t2, si, AF.Square,
                                             bias=b2_sb[:, 8 + fc:9 + fc])
                    else:
                        nc.scalar.activation(t2, bB[:, 0:256], AF.Square,
                                             bias=b2_sb[:, 8 + fc:9 + fc])
                    nc.vector.tensor_add(t1, t1, t2)
                    nc.scalar.activation(ampT[:, fc], t1, AF.Sqrt)
                    for bt in range(2):
                        bs = slice(bt * 128, (bt + 1) * 128)
                        # pg[0]/pg[1] share one PSUM bank: only the very first
                        # matmul may set start (a start wipes the whole bank)
                        nc.tensor.matmul(pg[bt], ampT[:, fc, bs], wgn_sb[:, fc],
                                         start=(fc == 0 and bt == 0),
                                         stop=(fc == 7))

            if _DEBUG_DUMP:
                nc.sync.dma_start(dbg_o1r, o1rT)
                nc.sync.dma_start(dbg_amp, ampT)

            scopeD.__exit__(None, None, None)
            scopeE = nc.named_scope("stageE_gate"); scopeE.__enter__()

            out_v = out_d.rearrange("(bt p) e -> bt p e", bt=2)
            # softplus/noise chain packed over both bt halves (fewer
            # instructions + activation-table switches on the serial tail)
            logits2 = stage.tile([128, 2, E], F32, tag="logits", bufs=1)
            if training:
                stdn2 = stage.tile([128, 2, E], F32, tag="stdn", bufs=1)
                # softplus(z) = ln(1 + exp(z))
                nc.scalar.activation(stdn2, pgt[:, :, 128:128 + E], AF.Exp)
                nc.vector.tensor_scalar_add(stdn2, stdn2, 1.0)
                nc.scalar.activation(stdn2, stdn2, AF.Ln)
                # (softplus(z)+eps0)*eps
                nc.vector.scalar_tensor_tensor(
                    stdn2, stdn2, float(NOISE_EPS), eps_sb,
                    op0=ADD, op1=MULT)
                nc.vector.tensor_tensor(logits2, pgt[:, :, 0:E], stdn2,
                                        op=ADD)
            else:
                nc.vector.tensor_copy(logits2, pgt[:, :, 0:E])
            # emit the two bt chains step-interleaved so Vector and Scalar
            # pipeline across halves on the serial tail
            tl = {}
            for bt in range(2):
                tl[bt] = {
                    k: stage.tile(shp, F32, tag=k, bufs=2, name=f"{k}{bt}")
                    for k, shp in (("top8", [128, 8]), ("negmax", [128, 1]),
                                   ("ex", [128, E]), ("msk", [128, E]),
                                   ("ssum", [128, 1]), ("rinv", [128, 1]),
                                   ("gates", [128, E]))
                }
            for bt in range(2):
                nc.vector.max(tl[bt]["top8"], logits2[:, bt])
            for bt in range(2):
                nc.vector.tensor_scalar(tl[bt]["negmax"], tl[bt]["top8"][:, 0:1],
                                        -1.0, None, op0=MULT)
            for bt in range(2):
                nc.scalar.activation(tl[bt]["ex"], logits2[:, bt], AF.Exp,
                                     bias=tl[bt]["negmax"])
                nc.vector.tensor_scalar(tl[bt]["msk"], logits2[:, bt],
                                        tl[bt]["top8"][:, 2:3], None,
                                        op0=mybir.AluOpType.is_ge)
            for bt in range(2):
                nc.vector.tensor_mul(tl[bt]["ex"], tl[bt]["ex"], tl[bt]["msk"])
            for bt in range(2):
                nc.vector.reduce_sum(out=tl[bt]["ssum"], in_=tl[bt]["ex"],
                                     axis=mybir.AxisListType.X)
            for bt in range(2):
                nc.vector.reciprocal(tl[bt]["rinv"], tl[bt]["ssum"])
            for bt in range(2):
                nc.vector.tensor_scalar(tl[bt]["gates"], tl[bt]["ex"],
                                        tl[bt]["rinv"], None, op0=MULT)
                nc.sync.dma_start(out_v[bt], tl[bt]["gates"])

            scopeE.__exit__(None, None, None)

    nc.compile()
    return nc


_PROGRAM_CACHE = {}


def _get_program(training: bool):
    key = bool(training)
    if key not in _PROGRAM_CACHE:
        _PROGRAM_CACHE[key] = _build_program(key)
    return _PROGRAM_CACHE[key]


def _prep_inputs(x, fc_w, fc_b, w1, b1, w2, b2, w_gate, w_noise, eps):
    f32 = np.float32
    f16 = np.float16

    # ---- weights/constants shared by all cores ----
    ll = np.arange(1, F, dtype=np.int64)[:, None]  # l' = 1..1023
    ff = np.arange(1, F + 1, dtype=np.int64)[None, :]
    ang = 2.0 * np.pi * ((ll * ff) % L).astype(np.float64) / L
    scale = 1.0 / np.sqrt(L)
    Ch = np.empty((F, F), np.float64)
    Sh = np.empty((F, F), np.float64)
    Ch[0, :] = scale
    Ch[1:, :] = np.cos(ang) * scale
    Sh[0, :] = 0.0
    Sh[1:, :] = -np.sin(ang) * scale
    # [p(l'), kc, fc, 128]
    chs = Ch.astype(f16).reshape(8, 128, 8, 128).transpose(1, 0, 2, 3)
    shs = Sh.astype(f16).reshape(8, 128, 8, 128).transpose(1, 0, 2, 3)

    # midpoint row: C[1024, f] = (-1)^f * scale, f = p+1 within each chunk
    p = np.arange(128)
    alt = (np.where((p + 1) % 2 == 0, 1.0, -1.0) * scale).astype(f16)
    alt = alt.reshape(1, 128)

    # quantize to fp16 FIRST, then combine: the shared per-weight
    # quantization errors cancel in the Karatsuba recombination, leaving
    # only a half-ulp residual on the combos (vs 3x error with combos
    # quantized independently)
    w1r = np.asarray(w1[0], f32).astype(f16).astype(f32)
    w1i = np.asarray(w1[1], f32).astype(f16).astype(f32)
    if KARA1:
        k1m, k2m, k3m = w1r, w1r + w1i, w1i - w1r
    else:
        k1m, k2m, k3m = w1r, w1i, w1i  # k3 unused

    def tile1(M):
        return np.ascontiguousarray(
            M.reshape(8, 128, 8, 512).transpose(2, 1, 0, 3).astype(f16))

    w2r = np.asarray(w2[0], f32).astype(f16).astype(f32)
    w2i = np.asarray(w2[1], f32).astype(f16).astype(f32)
    if KARA2:
        m1m, m2m, m3m = w2r, w2r + w2i, w2i - w2r
    else:
        m1m, m2m, m3m = w2r, w2i, w2i  # m3 unused

    def tile2(M):
        return np.ascontiguousarray(
            M.reshape(32, 128, 4, 256).transpose(2, 1, 0, 3).astype(f16))

    wgn = np.zeros((F, 256), f32)
    wgn[:, 0:E] = np.asarray(w_gate, f32)
    wgn[:, 128:128 + E] = np.asarray(w_noise, f32)
    wgn = rnd11(wgn).reshape(8, 128, 256).transpose(1, 0, 2)

    b1all = np.zeros((128, 64), f32)
    b1all[:, 0:32] = np.asarray(b1[0], f32).reshape(32, 128).T
    b1all[:, 32:64] = np.asarray(b1[1], f32).reshape(32, 128).T
    b2all = np.zeros((128, 16), f32)
    b2all[:, 0:8] = np.asarray(b2[0], f32).reshape(8, 128).T
    b2all[:, 8:16] = np.asarray(b2[1], f32).reshape(8, 128).T

    common = {
        "chs": np.ascontiguousarray(chs),
        "shs": np.ascontiguousarray(shs),
        "k1": tile1(k1m),
        "k2": tile1(k2m),
        "k3": tile1(k3m),
        "m1": tile2(m1m),
        "m2": tile2(m2m),
        "m3": tile2(m3m),
        "wgn": np.ascontiguousarray(wgn),
        "altrow": alt,
        "b1all": b1all,
        "b2all": b2all,
    }

    # ---- per-core data ----
    x = np.asarray(x, f32)
    fcw = np.asarray(fc_w, f32).reshape(CH)
    eps = np.asarray(eps, f32)

    in_maps = []
    for i in range(NCORES):
        xs = x[i * BL:(i + 1) * BL]  # [256, 2048, 16]
        # xw[b, c, l] = x[b, l, c] * fc_w[c] (scale folded host-side)
        xw = xs.transpose(0, 2, 1) * fcw[None, :, None]  # [256, 16, 2048]
        xe = np.empty((BL, CH, F), f32)
        xo = np.empty((BL, CH, F), f32)
        xe[:, :, 0] = xw[:, :, 0]
        xo[:, :, 0] = 0.0
        fwd = xw[:, :, 1:1024]
        rev = xw[:, :, 2047:1024:-1]
        xe[:, :, 1:1024] = fwd + rev
        xo[:, :, 1:1024] = fwd - rev
        hm = xw[:, :, 1024].sum(axis=1)  # [256]
        # tile: [eo, b, c, l'] -> [eo, lc, p(l'), c, b]
        xeo = np.stack([xe, xo])  # [2, 256, 16, 1024]
        xeo = xeo.transpose(0, 3, 2, 1).astype(f16)  # [2, 1024, 16, 256]
        sh = dict(common)
        sh["xeo"] = np.ascontiguousarray(xeo.reshape(2, 8, 128, CH, 256))
        sh["hmrow"] = hm.astype(f16).reshape(1, 256)
        esh = eps[i * BL:(i + 1) * BL]  # [256, E]
        sh["eps"] = np.ascontiguousarray(esh.reshape(2, 128, E).transpose(1, 0, 2))
        in_maps.append(sh)
    return in_maps


def run(inputs, trace=False):
    """Returns (gates [B, E] float32, BassKernelResults)."""
    x = np.asarray(inputs["x"], np.float32)
    fc_w = np.asarray(inputs["fc_w"], np.float32)
    fc_b = np.asarray(inputs["fc_b"], np.float32)
    w1 = np.asarray(inputs["w1"], np.float32)
    b1 = np.asarray(inputs["b1"], np.float32)
    w2 = np.asarray(inputs["w2"], np.float32)
    b2 = np.asarray(inputs["b2"], np.float32)
    w_gate = np.asarray(inputs["w_gate"], np.float32)
    w_noise = np.asarray(inputs["w_noise"], np.float32)
    eps = np.asarray(inputs["eps"], np.float32)
    training = bool(int(np.asarray(inputs.get("training", 1))))

    nc = _get_program(training)
    in_maps = _prep_inputs(x, fc_w, fc_b, w1, b1, w2, b2, w_gate, w_noise, eps)
    res = run_bass_kernel_spmd(
        nc, in_maps, core_ids=list(range(NCORES)), trace=trace,
    )
    gates = np.concatenate([r["out"] for r in res.results], axis=0)
    return gates.astype(np.float32), res


def kernel(**inputs):
    gates, _ = run(inputs, trace=False)
    return gates


# revision 42
# speedup vs baseline: 1.1132x; 1.0478x over previous
"""Trainium2 Bass kernel v3 for nn_AdaptiveFourierTransformGateLayer.

Data-parallel over batch: 8 cores x 256 rows. Per core:

  Host prep: xw = x * fc_w (scale+layout only), reflection-fold over l:
    xe[b,c,l'] = xw[b,l',c] + xw[b,2048-l',c]   (l'=1..1023; l'=0 -> xw[b,0,c])
    xo[b,c,l'] = xw[b,l',c] - xw[b,2048-l',c]   (l'=0 -> 0)
    hm[b] = sum_c xw[b,1024,c]                  (midpoint row)
  Folding halves the DFT to 1024x1024 half-matrices (C even / S odd).
  fc_b is dropped: AC-bin column sums of the DFT are exactly zero.
  Everything streamed in fp16, laid out [l'-part, c, b] so the channel
  tree-reduce lands directly in DFT-ready [l', b] layout (no transposes).

  Device:
  A: c-tree reduction (DVE, fp16 2x mode) -> HeT/HoT [l'-part, b].
  B: DFT chase: per f-chunk PSUM bank holds xr | xi halves; fp16 matmuls
     Ch-chunk^T @ HeT / Sh-chunk^T @ HoT accumulate as l'-chunks arrive.
     Midpoint rank-1 term alt(f) x hm(b) closes xr. 7 banks chase, f-chunk
     7 runs as a second wave after bank 0 evacuates. Evac to fp16
     xr/xi/xs (xs = xr+xi for Karatsuba).
  C: layer 1 via 3-matmul Karatsuba complex product:
       m1 = (xr+xi)@W1r, m2 = xi@(W1r+W1i), m3 = xr@(W1i-W1r)
       o1r = relu(m1-m2+b1r), o1i = relu(m1+m3+b1i), o1s = o1r+o1i
     m1|m2 share a PSUM bank, m3 in a second bank. Transposed dataflow
     (stationary = weight chunk, moving = activations [128,256]).
  D: layer 2 same Karatsuba shape; amp = sqrt((m1-m2+b2r)^2+(m1+m3+b2i)^2)
     -> ampT f32r. Gate matmuls (ampT @ wgn, f32r) chased per f-chunk.
  E: noisy top-3 softmax -> gates (small DVE/Act chain only).
"""
import sys
import types
import contextlib
import ctypes

import numpy as np

if "/opt/trn_rl_repo" not in sys.path:
    sys.path.insert(0, "/opt/trn_rl_repo")

# ---------------------------------------------------------------------------
# NTFF trace hook shim (only used when trace=True; harmless otherwise)
# ---------------------------------------------------------------------------


def _install_trace_shim():
    if "antenv.axon_hooks" in sys.modules:
        return
    so_path = "/opt/axon/libaxon_pjrt.so"

    def _mk():
        try:
            lib = ctypes.CDLL(so_path)
        except OSError:
            return None
        if not hasattr(lib, "axon_start_nrt_profile"):
            return None
        lib.axon_start_nrt_profile.argtypes = [
            ctypes.POINTER(ctypes.c_int64),
            ctypes.c_size_t,
        ]
        lib.axon_start_nrt_profile.restype = ctypes.c_int64
        lib.axon_stop_nrt_profile.argtypes = [ctypes.c_char_p]
        lib.axon_stop_nrt_profile.restype = ctypes.c_int64

        @contextlib.contextmanager
        def _hook(output_dir, device_ids):
            import jax

            jax.devices()
            if device_ids:
                ids = (ctypes.c_int64 * len(device_ids))(*device_ids)
                rc = lib.axon_start_nrt_profile(ids, len(device_ids))
            else:
                rc = lib.axon_start_nrt_profile(None, 0)
            if rc != 0:
                raise RuntimeError(f"axon_start_nrt_profile rc={rc}")
            try:
                yield
            finally:
                n = lib.axon_stop_nrt_profile(str(output_dir).encode())
                print(f"profile: {n} file(s) written to {output_dir}", file=sys.stderr)

        return _hook

    mod = types.ModuleType("antenv.axon_hooks")
    mod._hook = _mk()
    mod.get_axon_ntff_profile_hook = lambda: mod._hook
    mod.set_axon_ntff_profile_hook = lambda h: setattr(mod, "_hook", h)
    sys.modules["antenv.axon_hooks"] = mod
    try:
        import antenv

        antenv.axon_hooks = mod
    except ImportError:
        pass


_install_trace_shim()

import concourse.tile as tile  # noqa: E402
from concourse import bacc, mybir  # noqa: E402
from concourse.bass_utils import run_bass_kernel_spmd  # noqa: E402

# ---------------------------------------------------------------------------
# Problem constants (hardcoded)
# ---------------------------------------------------------------------------
B = 2048
L = 2048
CH = 16
F = 1024  # num freqs (rfft bins 1..1024)
FH = 4096  # hidden
E = 88  # num experts
NOISE_EPS = 0.01
_DEBUG_DUMP = False
NCORES = 8
BL = B // NCORES  # 256 rows per core
F32R = mybir.dt.float32r
F32 = mybir.dt.float32
FP16 = mybir.dt.float16

KARA1 = True  # Karatsuba in layer 1 (exact fp16 combos; revert to False
              # if the flip count pushes rel err near 2e-2)
KARA2 = True  # ... in layer 2

ADD = mybir.AluOpType.add
MULT = mybir.AluOpType.mult
AF = mybir.ActivationFunctionType


def rnd11(x):
    """Round-to-nearest keeping 11 mantissa bits (hardware f32r rounding)."""
    a = np.ascontiguousarray(x, np.float32)
    ai = a.view(np.uint32)
    return ((ai + np.uint32(1 << 11)) & np.uint32(0xFFFFF000)).view(np.float32)


def _build_program(training: bool):
    nc = bacc.Bacc("TRN2", target_bir_lowering=False, debug=False, num_devices=NCORES)

    # [eo, lc, p(l'), c, b] - host pre-transposed so tree-reduce -> [l', b]
    xeo_d = nc.dram_tensor("xeo", [2, 8, 128, CH, 256], FP16,
                           kind="ExternalInput").ap()
    # [p(l'), kc, fc, 128 f-cols]
    chs_d = nc.dram_tensor("chs", [128, 8, 8, 128], FP16, kind="ExternalInput").ap()
    shs_d = nc.dram_tensor("shs", [128, 8, 8, 128], FP16, kind="ExternalInput").ap()
    # [hg, p(f), fc, h-cols 512]
    k1_d = nc.dram_tensor("k1", [8, 128, 8, 512], FP16, kind="ExternalInput").ap()
    k2_d = nc.dram_tensor("k2", [8, 128, 8, 512], FP16, kind="ExternalInput").ap()
    k3_d = nc.dram_tensor("k3", [8, 128, 8, 512], FP16, kind="ExternalInput").ap()
    # [fp, p(h), hc, f-cols 256]
    m1_d = nc.dram_tensor("m1", [4, 128, 32, 256], FP16, kind="ExternalInput").ap()
    m2_d = nc.dram_tensor("m2", [4, 128, 32, 256], FP16, kind="ExternalInput").ap()
    m3_d = nc.dram_tensor("m3", [4, 128, 32, 256], FP16, kind="ExternalInput").ap()
    # [p(f), fc, 256] - cols 0:88 gate, 128:216 noise
    wgn_d = nc.dram_tensor("wgn", [128, 8, 256], F32R, kind="ExternalInput").ap()
    hm_d = nc.dram_tensor("hmrow", [1, 256], FP16, kind="ExternalInput").ap()
    alt_d = nc.dram_tensor("altrow", [1, 128], FP16, kind="ExternalInput").ap()
    b1_d = nc.dram_tensor("b1all", [128, 64], F32, kind="ExternalInput").ap()  # r|i
    b2_d = nc.dram_tensor("b2all", [128, 16], F32, kind="ExternalInput").ap()  # r|i
    eps_d = nc.dram_tensor("eps", [128, 2, E], F32, kind="ExternalInput").ap()
    out_d = nc.dram_tensor("out", [BL, E], F32, kind="ExternalOutput").ap()
    if _DEBUG_DUMP:
        dbg_het = nc.dram_tensor("dbg_het", [128, 8, 256], FP16, kind="ExternalOutput").ap()
        dbg_hot = nc.dram_tensor("dbg_hot", [128, 8, 256], FP16, kind="ExternalOutput").ap()
        dbg_xr = nc.dram_tensor("dbg_xr", [128, 8, 256], FP16, kind="ExternalOutput").ap()
        dbg_xi = nc.dram_tensor("dbg_xi", [128, 8, 256], FP16, kind="ExternalOutput").ap()
        dbg_o1r = nc.dram_tensor("dbg_o1r", [128, 32, 256], FP16, kind="ExternalOutput").ap()
        dbg_amp = nc.dram_tensor("dbg_amp", [128, 8, 256], F32R, kind="ExternalOutput").ap()

    with tile.TileContext(nc) as tc:
        with tc.tile_pool(name="consts", bufs=1) as consts, \
             tc.tile_pool(name="xstream", bufs=4) as xstream, \
             tc.tile_pool(name="wring", bufs=6) as wring, \
             tc.tile_pool(name="h8", bufs=1) as h8, \
             tc.tile_pool(name="acts", bufs=1) as acts, \
             tc.tile_pool(name="o16", bufs=1) as o16, \
             tc.tile_pool(name="stage", bufs=4) as stage, \
             tc.tile_pool(name="ps", bufs=1, space="PSUM") as ps:

            hm_sb = consts.tile([1, 256], FP16, tag="hm")
            nc.sync.dma_start(hm_sb, hm_d)
            alt_sb = consts.tile([1, 128], FP16, tag="alt")
            nc.sync.dma_start(alt_sb, alt_d)
            b1_sb = consts.tile([128, 64], F32, tag="b1")
            nc.sync.dma_start(b1_sb, b1_d)
            b2_sb = consts.tile([128, 16], F32, tag="b2")
            nc.sync.dma_start(b2_sb, b2_d)
            eps_sb = consts.tile([128, 2, E], F32, tag="eps")
            nc.sync.dma_start(eps_sb, eps_d)
            wgn_sb = consts.tile([128, 8, 256], F32R, tag="wgn")

            # DFT half-matrices on a dedicated tag (persist through stage A);
            # split into kc-halves so their DMA interleaves with the x stream
            cs_sb = {}
            for half in range(2):
                cs_sb[("c", half)] = wring.tile([128, 4, 8, 128], FP16,
                                                tag="cs", bufs=4,
                                                name=f"chs{half}")
                cs_sb[("s", half)] = wring.tile([128, 4, 8, 128], FP16,
                                                tag="cs", bufs=4,
                                                name=f"shs{half}")

            # persistent activations
            HeT = h8.tile([128, 8, 256], FP16, tag="he", name="HeT")
            HoT = h8.tile([128, 8, 256], FP16, tag="ho", name="HoT")
            xrT = acts.tile([128, 8, 256], FP16, tag="xr")
            xiT = acts.tile([128, 8, 256], FP16, tag="xi")
            # 3rd stream: xs = xr+xi (Karatsuba) or xin = -xi (plain)
            x3T = acts.tile([128, 8, 256], FP16, tag="x3")
            ampT = acts.tile([128, 8, 256], F32R, tag="amp")
            o1rT = o16.tile([128, 32, 256], FP16, tag="o1r", name="o1rT")
            o1iT = o16.tile([128, 32, 256], FP16, tag="o1i", name="o1iT")
            o1sT = None
            if KARA2:
                o1sT = o16.tile([128, 32, 256], FP16, tag="o1s", name="o1sT")

            # ---------------- Stage A + B ----------------
            scopeA = nc.named_scope("stageA_dft"); scopeA.__enter__()

            psB = {}

            def bbank(fc):
                psB[fc] = ps.tile([128, 512], F32, tag="bank", bufs=7,
                                  name=f"B{fc}")

            def b_mms(fc, lc):
                ch = cs_sb[("c", lc // 4)]
                sh = cs_sb[("s", lc // 4)]
                nc.tensor.matmul(psB[fc][:, 0:256], ch[:, lc % 4, fc],
                                 HeT[:, lc], start=(lc == 0), stop=False)
                nc.tensor.matmul(psB[fc][:, 256:512], sh[:, lc % 4, fc],
                                 HoT[:, lc], start=False, stop=(lc == 7))

            def b_mid(fc):
                nc.tensor.matmul(psB[fc][:, 0:256], alt_sb, hm_sb,
                                 start=False, stop=True)

            def b_evac(fc):
                nc.scalar.copy(xrT[:, fc], psB[fc][:, 0:256])
                nc.scalar.copy(xiT[:, fc], psB[fc][:, 256:512])
                if KARA1:
                    # one-PSUM-operand rule: xr is already in SBUF (fp16)
                    nc.vector.tensor_tensor(x3T[:, fc], xrT[:, fc],
                                            psB[fc][:, 256:512], op=ADD)
                else:
                    nc.vector.tensor_scalar(x3T[:, fc], psB[fc][:, 256:512],
                                            -1.0, None, op0=MULT)

            for fc in range(7):
                bbank(fc)
            for lc in range(8):
                for eo in range(2):
                    xa = xstream.tile([128, CH, 256], FP16, tag="big",
                                      name=f"x{eo}_{lc}")
                    nc.sync.dma_start(xa, xeo_d[eo][lc])
                    nc.vector.tensor_tensor(xa[:, 0:8], xa[:, 0:8],
                                            xa[:, 8:16], op=ADD)
                    nc.vector.tensor_tensor(xa[:, 0:4], xa[:, 0:4],
                                            xa[:, 4:8], op=ADD)
                    nc.vector.tensor_tensor(xa[:, 0:2], xa[:, 0:2],
                                            xa[:, 2:4], op=ADD)
                    dst = HeT if eo == 0 else HoT
                    nc.vector.tensor_tensor(dst[:, lc], xa[:, 0], xa[:, 1],
                                            op=ADD)
                # CS halves queue behind the first x chunks (so the x stream
                # gets the early bandwidth) but are emitted BEFORE the first
                # b_mms that read them — emission order defines dependencies
                if lc == 0:
                    nc.sync.dma_start(cs_sb[("c", 0)], chs_d[:, 0:4])
                    nc.sync.dma_start(cs_sb[("s", 0)], shs_d[:, 0:4])
                if lc == 2:
                    nc.sync.dma_start(cs_sb[("c", 1)], chs_d[:, 4:8])
                    nc.sync.dma_start(cs_sb[("s", 1)], shs_d[:, 4:8])
                for fc in range(7):
                    b_mms(fc, lc)
            for fc in range(7):
                b_mid(fc)
            b_evac(0)
            # wave 2: f-chunk 7 reuses the bank slot of f-chunk 0
            bbank(7)
            for lc in range(8):
                b_mms(7, lc)
            b_mid(7)
            for fc in range(1, 8):
                b_evac(fc)

            if _DEBUG_DUMP:
                nc.sync.dma_start(dbg_het, HeT)
                nc.sync.dma_start(dbg_hot, HoT)
                nc.sync.dma_start(dbg_xr, xrT)
                nc.sync.dma_start(dbg_xi, xiT)

            scopeA.__exit__(None, None, None)
            scopeC = nc.named_scope("stageC_l1"); scopeC.__enter__()

            # L2 weight tiles, hc-half-split 1 MB each. DMAs are hoisted into
            # stage C's window (DMA is otherwise idle in late C while stage D
            # alone would need ~290 GB/s). fp0's first tiles ride the idle
            # xstream slots (same 8 KB/partition shape as the x chunks).
            nmat2 = 3 if KARA2 else 2
            mw = {}
            mds = (m1_d, m2_d, m3_d)[:nmat2]

            def m_tile(fp, mi, h, pool, tag):
                t = pool.tile([128, 16, 256], FP16, tag=tag,
                              bufs=(4 if tag in ("cs", "big") else 6),
                              name=f"m{mi}g{fp}h{h}")
                nc.sync.dma_start(t, mds[mi][fp][:, h * 16:(h + 1) * 16])
                mw[(fp, mi, h)] = t

            for hg in range(8):
                k1 = wring.tile([128, 8, 512], FP16, tag="w", name=f"k1g{hg}")
                nc.sync.dma_start(k1, k1_d[hg])
                k2 = wring.tile([128, 8, 512], FP16, tag="w", name=f"k2g{hg}")
                nc.sync.dma_start(k2, k2_d[hg])
                k3 = None
                if KARA1:
                    k3 = wring.tile([128, 8, 512], FP16, tag="w",
                                    name=f"k3g{hg}")
                    nc.sync.dma_start(k3, k3_d[hg])
                for j in range(4):
                    hc = hg * 4 + j
                    bA = ps.tile([128, 512], F32, tag="bank", bufs=7,
                                 name=f"cA{hc}")
                    bB = None
                    if KARA1:
                        bB = ps.tile([128, 512], F32, tag="bank", bufs=7,
                                     name=f"cB{hc}")
                    hsl = slice(j * 128, (j + 1) * 128)
                    for fc in range(8):
                        f0 = fc == 0
                        fl_ = fc == 7
                        if KARA1:
                            # m1 = (xr+xi)@W1r ; m2 = xi@(W1r+W1i)
                            # m3 = xr@(W1i-W1r)
                            nc.tensor.matmul(bA[:, 0:256], k1[:, fc, hsl],
                                             x3T[:, fc], start=f0, stop=fl_)
                            nc.tensor.matmul(bA[:, 256:512], k2[:, fc, hsl],
                                             xiT[:, fc], start=False, stop=fl_)
                            nc.tensor.matmul(bB[:, 0:256], k3[:, fc, hsl],
                                             xrT[:, fc], start=f0, stop=fl_)
                        else:
                            # o1r = xr@W1r + (-xi)@W1i ; o1i = xi@W1r + xr@W1i
                            nc.tensor.matmul(bA[:, 0:256], k1[:, fc, hsl],
                                             xrT[:, fc], start=f0, stop=False)
                            nc.tensor.matmul(bA[:, 0:256], k2[:, fc, hsl],
                                             x3T[:, fc], start=False, stop=fl_)
                            nc.tensor.matmul(bA[:, 256:512], k1[:, fc, hsl],
                                             xiT[:, fc], start=False, stop=False)
                            nc.tensor.matmul(bA[:, 256:512], k2[:, fc, hsl],
                                             xrT[:, fc], start=False, stop=fl_)
                    if KARA1:
                        # one-PSUM-operand rule: stage m1 through SBUF
                        tm = stage.tile([128, 256], F32, tag="d", bufs=4,
                                        name=f"tm_{hc}")
                        nc.scalar.copy(tm, bA[:, 0:256])
                        d1 = stage.tile([128, 256], F32, tag="d", bufs=4,
                                        name=f"d1_{hc}")
                        nc.vector.tensor_sub(d1, tm, bA[:, 256:512])
                        nc.scalar.activation(o1rT[:, hc], d1, AF.Relu,
                                             bias=b1_sb[:, hc:hc + 1])
                        d2 = stage.tile([128, 256], F32, tag="d", bufs=4,
                                        name=f"d2_{hc}")
                        nc.vector.tensor_add(d2, tm, bB[:, 0:256])
                        nc.scalar.activation(o1iT[:, hc], d2, AF.Relu,
                                             bias=b1_sb[:, 32 + hc:33 + hc])
                    else:
                        nc.scalar.activation(o1rT[:, hc], bA[:, 0:256], AF.Relu,
                                             bias=b1_sb[:, hc:hc + 1])
                        nc.scalar.activation(o1iT[:, hc], bA[:, 256:512],
                                             AF.Relu,
                                             bias=b1_sb[:, 32 + hc:33 + hc])
                    if KARA2:
                        nc.vector.tensor_tensor(o1sT[:, hc], o1rT[:, hc],
                                                o1iT[:, hc], op=ADD)

            scopeC.__exit__(None, None, None)
            scopeD = nc.named_scope("stageD_l2"); scopeD.__enter__()

            nc.sync.dma_start(wgn_sb, wgn_d)

            pgt = ps.tile([128, 2, 256], F32, tag="pg", bufs=1, name="pg")
            pg = [pgt[:, 0], pgt[:, 1]]

            for fp in range(4):
                for h in range(2):
                    for mi in range(nmat2):
                        m_tile(fp, mi, h, wring, "w")
                for fl in range(2):
                    fc = fp * 2 + fl
                    bA = ps.tile([128, 512], F32, tag="bank", bufs=7,
                                 name=f"dA{fc}")
                    bB = ps.tile([128, 512], F32, tag="bank", bufs=7,
                                 name=f"dB{fc}")
                    fsl = slice(fl * 128, (fl + 1) * 128)
                    for hc in range(32):
                        h0 = hc == 0
                        hl = hc == 31
                        hh = hc // 16
                        hr = hc % 16
                        m1 = mw[(fp, 0, hh)]
                        m2 = mw[(fp, 1, hh)]
                        if KARA2:
                            m3 = mw[(fp, 2, hh)]
                            # m1 = o1s@W2r ; m2 = o1i@(W2r+W2i)
                            # m3 = o1r@(W2i-W2r)
                            nc.tensor.matmul(bA[:, 0:256], m1[:, hr, fsl],
                                             o1sT[:, hc], start=h0, stop=hl)
                            nc.tensor.matmul(bA[:, 256:512], m2[:, hr, fsl],
                                             o1iT[:, hc], start=False, stop=hl)
                            nc.tensor.matmul(bB[:, 0:256], m3[:, hr, fsl],
                                             o1rT[:, hc], start=h0, stop=hl)
                        else:
                            # qa|qb in bA, qi in bB:
                            # o2r = qa - qb ; o2i = qi
                            nc.tensor.matmul(bA[:, 0:256], m1[:, hr, fsl],
                                             o1rT[:, hc], start=h0, stop=hl)
                            nc.tensor.matmul(bA[:, 256:512], m2[:, hr, fsl],
                                             o1iT[:, hc], start=False, stop=hl)
                            nc.tensor.matmul(bB[:, 0:256], m1[:, hr, fsl],
                                             o1iT[:, hc], start=h0, stop=False)
                            nc.tensor.matmul(bB[:, 0:256], m2[:, hr, fsl],
                                             o1rT[:, hc], start=False, stop=hl)
                    # one-PSUM-operand rule: stage m1 through SBUF first
                    tm = stage.tile([128, 256], F32, tag="d", bufs=4,
                                    name=f"tm{fc}")
                    nc.scalar.copy(tm, bA[:, 0:256])
                    sd = stage.tile([128, 256], F32, tag="d", bufs=4,
                                    name=f"sd{fc}")
                    nc.vector.tensor_sub(sd, tm, bA[:, 256:512])
                    t1 = stage.tile([128, 256], F32, tag="d", bufs=4,
                                    name=f"sqr{fc}")
                    nc.scalar.activation(t1, sd, AF.Square,
                                         bias=b2_sb[:, fc:fc + 1])
                    t2 = stage.tile([128, 256], F32, tag="d", bufs=4,
                                    name=f"sqi{fc}")
                    if KARA2:
                        si = stage.tile([128, 256], F32, tag="d", bufs=4,
                                        name=f"si{fc}")
                        nc.vector.tensor_add(si, tm, bB[:, 0:256])
                        nc.scalar.activation(t2, si, AF.Square,
                                             bias=b2_sb[:, 8 + fc:9 + fc])
                    else:
                        nc.scalar.activation(t2, bB[:, 0:256], AF.Square,
                                             bias=b2_sb[:, 8 + fc:9 + fc])
                    nc.vector.tensor_add(t1, t1, t2)
                    nc.scalar.activation(ampT[:, fc], t1, AF.Sqrt)
                    for bt in range(2):
                        bs = slice(bt * 128, (bt + 1) * 128)
                        # pg[0]/pg[1] share one PSUM bank: only the very first
                        # matmul may set start (a start wipes the whole bank)
                        nc.tensor.matmul(pg[bt], ampT[:, fc, bs], wgn_sb[:, fc],
                                         start=(fc == 0 and bt == 0),
                                         stop=(fc == 7))

            if _DEBUG_DUMP:
                nc.sync.dma_start(dbg_o1r, o1rT)
                nc.sync.dma_start(dbg_amp, ampT)

            scopeD.__exit__(None, None, None)
            scopeE = nc.named_scope("stageE_gate"); scopeE.__enter__()

            out_v = out_d.rearrange("(bt p) e -> bt p e", bt=2)
            # softplus/noise chain packed over both bt halves (fewer
            # instructions + activation-table switches on the serial tail)
            logits2 = stage.tile([128, 2, E], F32, tag="logits", bufs=1)
            if training:
                stdn2 = stage.tile([128, 2, E], F32, tag="stdn", bufs=1)
                # softplus(z) = ln(1 + exp(z))
                nc.scalar.activation(stdn2, pgt[:, :, 128:128 + E], AF.Exp)
                nc.vector.tensor_scalar_add(stdn2, stdn2, 1.0)
                nc.scalar.activation(stdn2, stdn2, AF.Ln)
                # (softplus(z)+eps0)*eps
                nc.vector.scalar_tensor_tensor(
                    stdn2, stdn2, float(NOISE_EPS), eps_sb,
                    op0=ADD, op1=MULT)
                nc.vector.tensor_tensor(logits2, pgt[:, :, 0:E], stdn2,
                                        op=ADD)
            else:
                nc.vector.tensor_copy(logits2, pgt[:, :, 0:E])
            # emit the two bt chains step-interleaved so Vector and Scalar
            # pipeline across halves on the serial tail
            tl = {}
            for bt in range(2):
                tl[bt] = {
                    k: stage.tile(shp, F32, tag=k, bufs=2, name=f"{k}{bt}")
                    for k, shp in (("top8", [128, 8]), ("negmax", [128, 1]),
                                   ("ex", [128, E]), ("msk", [128, E]),
                                   ("ssum", [128, 1]), ("rinv", [128, 1]),
                                   ("gates", [128, E]))
                }
            for bt in range(2):
                nc.vector.max(tl[bt]["top8"], logits2[:, bt])
            for bt in range(2):
                nc.vector.tensor_scalar(tl[bt]["negmax"], tl[bt]["top8"][:, 0:1],
                                        -1.0, None, op0=MULT)
            for bt in range(2):
                nc.scalar.activation(tl[bt]["ex"], logits2[:, bt], AF.Exp,
                                     bias=tl[bt]["negmax"])
                nc.vector.tensor_scalar(tl[bt]["msk"], logits2[:, bt],
                                        tl[bt]["top8"][:, 2:3], None,
                                        op0=mybir.AluOpType.is_ge)
            for bt in range(2):
                nc.vector.tensor_mul(tl[bt]["ex"], tl[bt]["ex"], tl[bt]["msk"])
            for bt in range(2):
                nc.vector.reduce_sum(out=tl[bt]["ssum"], in_=tl[bt]["ex"],
                                     axis=mybir.AxisListType.X)
            for bt in range(2):
                nc.vector.reciprocal(tl[bt]["rinv"], tl[bt]["ssum"])
            for bt in range(2):
                nc.vector.tensor_scalar(tl[bt]["gates"], tl[bt]["ex"],
                                        tl[bt]["rinv"], None, op0=MULT)
                nc.sync.dma_start(out_v[bt], tl[bt]["gates"])

            scopeE.__exit__(None, None, None)

    nc.compile()
    return nc


_PROGRAM_CACHE = {}


def _get_program(training: bool):
    key = bool(training)
    if key not in _PROGRAM_CACHE:
        _PROGRAM_CACHE[key] = _build_program(key)
    return _PROGRAM_CACHE[key]


def _prep_inputs(x, fc_w, fc_b, w1, b1, w2, b2, w_gate, w_noise, eps):
    f32 = np.float32
    f16 = np.float16

    # ---- weights/constants shared by all cores ----
    ll = np.arange(1, F, dtype=np.int64)[:, None]  # l' = 1..1023
    ff = np.arange(1, F + 1, dtype=np.int64)[None, :]
    ang = 2.0 * np.pi * ((ll * ff) % L).astype(np.float64) / L
    scale = 1.0 / np.sqrt(L)
    Ch = np.empty((F, F), np.float64)
    Sh = np.empty((F, F), np.float64)
    Ch[0, :] = scale
    Ch[1:, :] = np.cos(ang) * scale
    Sh[0, :] = 0.0
    Sh[1:, :] = -np.sin(ang) * scale
    # [p(l'), kc, fc, 128]
    chs = Ch.astype(f16).reshape(8, 128, 8, 128).transpose(1, 0, 2, 3)
    shs = Sh.astype(f16).reshape(8, 128, 8, 128).transpose(1, 0, 2, 3)

    # midpoint row: C[1024, f] = (-1)^f * scale, f = p+1 within each chunk
    p = np.arange(128)
    alt = (np.where((p + 1) % 2 == 0, 1.0, -1.0) * scale).astype(f16)
    alt = alt.reshape(1, 128)

    # quantize to fp16 FIRST, then combine: the shared per-weight
    # quantization errors cancel in the Karatsuba recombination, leaving
    # only a half-ulp residual on the combos (vs 3x error with combos
    # quantized independently)
    w1r = np.asarray(w1[0], f32).astype(f16).astype(f32)
    w1i = np.asarray(w1[1], f32).astype(f16).astype(f32)
    if KARA1:
        k1m, k2m, k3m = w1r, w1r + w1i, w1i - w1r
    else:
        k1m, k2m, k3m = w1r, w1i, w1i  # k3 unused

    def tile1(M):
        return np.ascontiguousarray(
            M.reshape(8, 128, 8, 512).transpose(2, 1, 0, 3).astype(f16))

    w2r = np.asarray(w2[0], f32).astype(f16).astype(f32)
    w2i = np.asarray(w2[1], f32).astype(f16).astype(f32)
    if KARA2:
        m1m, m2m, m3m = w2r, w2r + w2i, w2i - w2r
    else:
        m1m, m2m, m3m = w2r, w2i, w2i  # m3 unused

    def tile2(M):
        return np.ascontiguousarray(
            M.reshape(32, 128, 4, 256).transpose(2, 1, 0, 3).astype(f16))

    wgn = np.zeros((F, 256), f32)
    wgn[:, 0:E] = np.asarray(w_gate, f32)
    wgn[:, 128:128 + E] = np.asarray(w_noise, f32)
    wgn = rnd11(wgn).reshape(8, 128, 256).transpose(1, 0, 2)

    b1all = np.zeros((128, 64), f32)
    b1all[:, 0:32] = np.asarray(b1[0], f32).reshape(32, 128).T
    b1all[:, 32:64] = np.asarray(b1[1], f32).reshape(32, 128).T
    b2all = np.zeros((128, 16), f32)
    b2all[:, 0:8] = np.asarray(b2[0], f32).reshape(8, 128).T
    b2all[:, 8:16] = np.asarray(b2[1], f32).reshape(8, 128).T

    common = {
        "chs": np.ascontiguousarray(chs),
        "shs": np.ascontiguousarray(shs),
        "k1": tile1(k1m),
        "k2": tile1(k2m),
        "k3": tile1(k3m),
        "m1": tile2(m1m),
        "m2": tile2(m2m),
        "m3": tile2(m3m),
        "wgn": np.ascontiguousarray(wgn),
        "altrow": alt,
        "b1all": b1all,
        "b2all": b2all,
    }

    # ---- per-core data ----
    x = np.asarray(x, f32)
    fcw = np.asarray(fc_w, f32).reshape(CH)
    eps = np.asarray(eps, f32)

    in_maps = []
    for i in range(NCORES):
        xs = x[i * BL:(i + 1) * BL]  # [256, 2048, 16]
        # xw[b, c, l] = x[b, l, c] * fc_w[c] (scale folded host-side)
        xw = xs.transpose(0, 2, 1) * fcw[None, :, None]  # [256, 16, 2048]
        xe = np.empty((BL, CH, F), f32)
        xo = np.empty((BL, CH, F), f32)
        xe[:, :, 0] = xw[:, :, 0]
        xo[:, :, 0] = 0.0
        fwd = xw[:, :, 1:1024]
        rev = xw[:, :, 2047:1024:-1]
        xe[:, :, 1:1024] = fwd + rev
        xo[:, :, 1:1024] = fwd - rev
        hm = xw[:, :, 1024].sum(axis=1)  # [256]
        # tile: [eo, b, c, l'] -> [eo, lc, p(l'), c, b]
        xeo = np.stack([xe, xo])  # [2, 256, 16, 1024]
        xeo = xeo.transpose(0, 3, 2, 1).astype(f16)  # [2, 1024, 16, 256]
        sh = dict(common)
        sh["xeo"] = np.ascontiguousarray(xeo.reshape(2, 8, 128, CH, 256))
        sh["hmrow"] = hm.astype(f16).reshape(1, 256)
        esh = eps[i * BL:(i + 1) * BL]  # [256, E]
        sh["eps"] = np.ascontiguousarray(esh.reshape(2, 128, E).transpose(1, 0, 2))
        in_maps.append(sh)
    return in_maps


def run(inputs, trace=False):
    """Returns (gates [B, E] float32, BassKernelResults)."""
    x = np.asarray(inputs["x"], np.float32)
    fc_w = np.asarray(inputs["fc_w"], np.float32)
    fc_b = np.asarray(inputs["fc_b"], np.float32)
    w1 = np.asarray(inputs["w1"], np.float32)
    b1 = np.asarray(inputs["b1"], np.float32)
    w2 = np.asarray(inputs["w2"], np.float32)
    b2 = np.asarray(inputs["b2"], np.float32)
    w_gate = np.asarray(inputs["w_gate"], np.float32)
    w_noise = np.asarray(inputs["w_noise"], np.float32)
    eps = np.asarray(inputs["eps"], np.float32)
    training = bool(int(np.asarray(inputs.get("training", 1))))

    nc = _get_program(training)
    in_maps = _prep_inputs(x, fc_w, fc_b, w1, b1, w2, b2, w_gate, w_noise, eps)
    res = run_bass_kernel_spmd(
        nc, in_maps, core_ids=list(range(NCORES)), trace=trace,
    )
    gates = np.concatenate([r["out"] for r in res.results], axis=0)
    return gates.astype(np.float32), res


def kernel(**inputs):
    gates, _ = run(inputs, trace=False)
    return gates
